# revision 1
# baseline (speedup 1.0000x reference)
"""BlockSparseRingMultiheadDilatedAttention Trainium2 kernel.

Problem (hardcoded): B=1, N=8192, E=1024, H=16 heads, D=64.
Two dilated groups: g0 = heads 0-7, seg 2048, dilation 1;
                    g1 = heads 8-15, seg 4096, dilation 2, offset 1 (odd positions).
Causal within each (gathered) segment.

Sharding over 8 cores (uniform SPMD program, per-core data):
  core c: a = c%2, sc = c//2, b = c%4, rc = c//4
    g0: seg sc (rows 2048*sc .. +2048), heads 4a..4a+4   (4 blocks of [2048 x 2048])
    g1: seg rc odd rows (gathered, 2048 rows), heads 8+2b..+2 (2 blocks)
  Host pre-slices inputs (bf16 cast, odd-row gather, weight head slices) so the
  device program is identical on every core.  Host sums the per-core partial
  output projections (disjoint head contributions) and adds bo.

Device dataflow per core (host pre-transposes X/W so all DMAs are plain
contiguous loads):
  - X^T / W^T tiles (embedding on partitions) loaded with large plain DMAs
  - qT/kT/vT = W^T-stationary matmuls (heads packed in pairs -> K=128, M=128),
    accumulated in paired 2-bank PSUM tiles, one ACT copy+bias per [128,1024]
  - V natural via PE transpose of vT, with an appended ones column
  - S^T tile pairs = K Q^T (per head) into 2-bank PSUM; ONE exp per [128,1024]
    on ACT (scale=1/8, bf16 out); causal masking via DVE mul with 0/1 masks on
    the 2 diagonal pairs per tq chunk; S-pair(i+1) emitted before O-pair(i)
    so the PE has runway while ACT computes the exp
  - O'^T accumulation = [V|1]^T P^T in PSUM; row 64 = softmax denominators
  - normalize via DVE reciprocal + gpsimd partition_broadcast + DVE mul (reads
    PSUM directly) -> O^T (bf16)
  - y = sum_pairs (O^T_pair)^T @ Wo^T_pair into 2-bank PSUM, fp32 out
"""

import numpy as np
import ml_dtypes

BF16 = ml_dtypes.bfloat16

SEG = 2048          # rows per attention block (both groups, post-gather)
E = 1024            # embedding
NQ = 512            # tq chunk (one PSUM bank of fp32)
NTQ = SEG // NQ     # 4 tq chunks per block
NTK = SEG // 128    # 16 tk chunks per block
ECH = E // 128      # 8 embedding chunks

_CACHE = {}


def _build_program():
    import concourse.bacc as bacc
    import concourse.mybir as mybir
    import concourse.tile as tile

    dt = mybir.dt
    nc = bacc.Bacc("TRN2", target_bir_lowering=False, debug=False,
                   enable_asserts=False)

    # ---- DRAM I/O (uniform across cores; host slices per core) ----
    xs = {}
    for sel in ("a", "b"):      # a = g0 rows, b = g1 gathered odd rows
        for inp in ("q", "k", "v"):
            xs[(sel, inp)] = nc.dram_tensor(
                f"x{sel}_{inp}", [E, SEG], dt.bfloat16, kind="ExternalInput").ap()
    ws = {inp: nc.dram_tensor(f"w{inp}", [E, 384], dt.bfloat16,
                              kind="ExternalInput").ap()
          for inp in ("q", "k", "v")}
    wo = nc.dram_tensor("wo", [384, E], dt.bfloat16, kind="ExternalInput").ap()
    bs = {inp: nc.dram_tensor(f"b{inp}", [384, 1], dt.float32,
                              kind="ExternalInput").ap()
          for inp in ("q", "k", "v")}
    y0 = nc.dram_tensor("y0", [SEG, E], dt.float32, kind="ExternalOutput").ap()
    y1 = nc.dram_tensor("y1", [SEG, E], dt.float32, kind="ExternalOutput").ap()

    with tile.TileContext(nc) as tc:
        from contextlib import ExitStack
        with ExitStack() as ctx:
            const = ctx.enter_context(tc.tile_pool(name="const", bufs=1))
            wpool = ctx.enter_context(tc.tile_pool(name="wpool", bufs=1))
            xtp = ctx.enter_context(tc.tile_pool(name="xtp", bufs=2))
            qkt = ctx.enter_context(tc.tile_pool(name="qkt", bufs=1))
            vtp = ctx.enter_context(tc.tile_pool(name="vtp", bufs=2))
            vnat = ctx.enter_context(tc.tile_pool(name="vnat", bufs=1))
            otp = ctx.enter_context(tc.tile_pool(name="otp", bufs=1))
            ptp = ctx.enter_context(tc.tile_pool(name="ptp", bufs=4))
            smallp = ctx.enter_context(tc.tile_pool(name="smallp", bufs=4))
            ypool = ctx.enter_context(tc.tile_pool(name="ypool", bufs=4))
            ps_mm = ctx.enter_context(
                tc.tile_pool(name="ps_mm", bufs=3, space="PSUM"))
            ps_acc = ctx.enter_context(
                tc.tile_pool(name="ps_acc", bufs=2, space="PSUM"))

            # ---- constants: identity (PE transpose), causal tile masks ----
            ident = const.tile([128, 128], dt.bfloat16, tag="ident")
            nc.gpsimd.memset(ident, 1.0)
            nc.gpsimd.affine_select(
                out=ident, in_=ident, compare_op=mybir.AluOpType.is_equal,
                fill=0.0, base=0, pattern=[[-1, 128]], channel_multiplier=1)
            masks2 = []
            for di0 in (0, 2):
                m = const.tile([128, 2 * NQ], dt.bfloat16, tag=f"mask{di0}")
                nc.gpsimd.memset(m, 1.0)
                # half hh covers di = di0 + hh: keep where f - p - 128*di >= 0
                nc.gpsimd.affine_select(
                    out=m.rearrange("p (hh f) -> p hh f", hh=2),
                    in_=m.rearrange("p (hh f) -> p hh f", hh=2),
                    compare_op=mybir.AluOpType.is_ge,
                    fill=0.0, base=-128 * di0, pattern=[[-128, 2], [1, NQ]],
                    channel_multiplier=-1)
                masks2.append(m)

            # ---- weights (host-pre-transposed; plain DMA loads) ----
            # wTa[inp]: [128 e, ECH*384]; chunk ec pair p at 384*ec + 128*p
            wTa = {}
            for inp in ("q", "k", "v"):
                t = wpool.tile([128, 384 * ECH], dt.bfloat16,
                               tag=f"wT_{inp}", name=f"wT_{inp}")
                for ec in range(ECH):
                    nc.sync.dma_start(
                        out=t[:, 384 * ec:384 * (ec + 1)],
                        in_=ws[inp][128 * ec:128 * (ec + 1), :])
                wTa[inp] = t
            # wT[inp][p]: 3D view [128 e, ECH, 128 d]; chunk ec = [:, ec, :]
            wT = {inp: [wTa[inp].rearrange("p (ec x) -> p ec x", x=384)
                        [:, :, 128 * p:128 * (p + 1)]
                        for p in range(3)] for inp in ("q", "k", "v")}
            # woT[p]: [128 (2 heads d), 1024 j]
            woT = []
            for p in range(3):
                t = wpool.tile([128, E], dt.bfloat16, tag=f"woT_{p}")
                nc.sync.dma_start(out=t, in_=wo[128 * p:128 * (p + 1), :])
                woT.append(t)
            # biases -> SBUF [128,1] per (inp, pair)
            bsb = {}
            for inp in ("q", "k", "v"):
                for p in range(3):
                    t = wpool.tile([128, 1], dt.float32, tag=f"b_{inp}_{p}")
                    nc.sync.dma_start(
                        out=t, in_=bs[inp][128 * p:128 * (p + 1), :])
                    bsb[(inp, p)] = t

            # ---- persistent per-pair activations ----
            qT = [qkt.tile([128, SEG], dt.bfloat16, tag=f"qT{p}", name=f"qT{p}")
                  for p in range(3)]
            kT = [qkt.tile([128, SEG], dt.bfloat16, tag=f"kT{p}", name=f"kT{p}")
                  for p in range(3)]
            # V natural + ones col: per pair [128, 16*130]; chunk i at 130*i,
            # head h lhsT = [:, 130*i + 65*h : +65]
            vn = [vnat.tile([128, NTK * 130], dt.bfloat16, tag=f"vn{p}", name=f"vn{p}")
                  for p in range(3)]
            oT = [otp.tile([128, SEG], dt.bfloat16, tag=f"oT{p}", name=f"oT{p}")
                  for p in range(3)]

            # ---- Phase A: projections ----
            for sel in ("a", "b"):
                pairs = (0, 1) if sel == "a" else (2,)
                for inp in ("q", "k", "v"):
                    xt = xtp.tile([128, ECH * SEG], dt.bfloat16, tag="xt")
                    for ec in range(ECH):
                        nc.sync.dma_start(
                            out=xt[:, SEG * ec:SEG * (ec + 1)],
                            in_=xs[(sel, inp)][128 * ec:128 * (ec + 1), :])
                    for p in pairs:
                        if inp == "v":
                            dst = vtp.tile([128, SEG], dt.bfloat16, tag="vT")
                        else:
                            dst = (qT if inp == "q" else kT)[p]
                        for t2 in range(NTQ // 2):
                            acc = ps_mm.tile([128, 2 * NQ], dt.float32,
                                             tag="mm")
                            for half in range(2):
                                for ec in range(ECH):
                                    nc.tensor.matmul(
                                        acc[:, NQ * half:NQ * (half + 1)],
                                        wT[inp][p][:, ec, :],
                                        xt[:, SEG * ec + NQ * (2 * t2 + half):
                                           SEG * ec + NQ * (2 * t2 + half + 1)],
                                        start=(ec == 0), stop=(ec == ECH - 1))
                            nc.scalar.activation(
                                dst[:, 2 * NQ * t2:2 * NQ * (t2 + 1)], acc,
                                mybir.ActivationFunctionType.Identity,
                                bias=bsb[(inp, p)], scale=1.0)
                        if inp == "v":
                            # transpose vT -> V natural (+ ones columns)
                            for i in range(NTK):
                                ptr = ps_acc.tile([128, 128], dt.bfloat16,
                                                  tag="acc", name="ptr")
                                nc.tensor.transpose(
                                    ptr, dst[:, 128 * i:128 * (i + 1)], ident)
                                dsts = vn[p][:, 130 * i:130 * i + 130]
                                dv = dsts.rearrange("p (h x) -> p h x", h=2)
                                nc.vector.tensor_copy(
                                    dv[:, :, 0:64],
                                    ptr.rearrange("p (h d) -> p h d", h=2))
                            ones_view = vn[p].rearrange(
                                "p (k x) -> p k x", x=130)
                            nc.gpsimd.memset(ones_view[:, :, 64:65], 1.0)
                            nc.gpsimd.memset(ones_view[:, :, 129:130], 1.0)

            # ---- Phase B: attention blocks ----
            # Pairs of tk-chunks share one 2-bank PSUM tile and one exp.
            # Emission is software-pipelined: S-pair(i2+1) is emitted before
            # O-pair(i2) so the PE has runway while ACT computes the exp.
            def attention_pair(p):
                for h in range(2):
                    hp = 64 * h
                    for j in range(NTQ):
                        n2 = 2 * (j + 1)
                        acc_o = ps_acc.tile([128, NQ], dt.float32, tag="acc")

                        def s_pair(i2):
                            s = ps_mm.tile([128, 2 * NQ], dt.float32,
                                           tag="mm", name="s")
                            for half in range(2):
                                i = 2 * i2 + half
                                nc.tensor.matmul(
                                    s[:, NQ * half:NQ * (half + 1)],
                                    kT[p][hp:hp + 64, 128 * i:128 * (i + 1)],
                                    qT[p][hp:hp + 64, NQ * j:NQ * (j + 1)],
                                    start=True, stop=True)
                            return s

                        def exp_pair(i2, s):
                            pt = ptp.tile([128, 2 * NQ], dt.bfloat16,
                                          tag="pt", name="pt")
                            nc.scalar.activation(
                                pt, s, mybir.ActivationFunctionType.Exp,
                                bias=0.0, scale=0.125)
                            if i2 >= 2 * j:       # diagonal pair
                                nc.vector.tensor_mul(
                                    pt, pt, masks2[i2 - 2 * j])
                            return pt

                        def o_pair(i2, pt):
                            for half in range(2):
                                i = 2 * i2 + half
                                nc.tensor.matmul(
                                    acc_o[0:65, :],
                                    vn[p][:, 130 * i + 65 * h:
                                          130 * i + 65 * h + 65],
                                    pt[:, NQ * half:NQ * (half + 1)],
                                    start=(i == 0), stop=(i == n2 * 2 - 1))

                        prev = None
                        for i2 in range(n2):
                            s = s_pair(i2)
                            if prev is not None:
                                o_pair(prev[0], prev[1])
                            prev = (i2, exp_pair(i2, s))
                        o_pair(prev[0], prev[1])

                        rj = smallp.tile([1, NQ], dt.float32, tag="rj",
                                         name="rj")
                        nc.vector.reciprocal(rj, acc_o[64:65, :])
                        rb = smallp.tile([64, NQ], dt.float32, tag="rb",
                                         name="rb")
                        nc.gpsimd.partition_broadcast(rb, rj)
                        nc.vector.tensor_mul(
                            oT[p][hp:hp + 64, NQ * j:NQ * (j + 1)],
                            acc_o[0:64, :], rb)

            # ---- Phase C: output projection ----
            def out_proj(ydram, pairs):
                for m in range(NTK):
                    accy = ps_mm.tile([128, 2 * NQ], dt.float32, tag="mm",
                                      name="accy")
                    for jc in range(2):
                        for idx, p in enumerate(pairs):
                            nc.tensor.matmul(
                                accy[:, NQ * jc:NQ * (jc + 1)],
                                oT[p][:, 128 * m:128 * (m + 1)],
                                woT[p][:, NQ * jc:NQ * (jc + 1)],
                                start=(idx == 0), stop=(idx == len(pairs) - 1))
                    ysb = ypool.tile([128, 2 * NQ], dt.float32, tag="ysb")
                    nc.vector.tensor_copy(ysb, accy)
                    nc.sync.dma_start(
                        out=ydram[128 * m:128 * (m + 1), :], in_=ysb)

            attention_pair(0)
            attention_pair(1)
            attention_pair(2)
            out_proj(y0, (0, 1))
            out_proj(y1, (2,))

    nc.compile()
    return nc


def _get_program():
    if "nc" not in _CACHE:
        _CACHE["nc"] = _build_program()
    return _CACHE["nc"]


def _prep_inputs(query, key, value, Wq, bq, Wk, bk, Wv, bv, Wo, bo):
    """Build the 8 per-core input maps (host-side slicing + bf16 cast)."""
    q = np.asarray(query, np.float32).reshape(8192, 1024).astype(BF16)
    k = np.asarray(key, np.float32).reshape(8192, 1024).astype(BF16)
    v = np.asarray(value, np.float32).reshape(8192, 1024).astype(BF16)
    wq = np.asarray(Wq, np.float32).astype(BF16)
    wk = np.asarray(Wk, np.float32).astype(BF16)
    wv = np.asarray(Wv, np.float32).astype(BF16)
    wo_f = np.asarray(Wo, np.float32).astype(BF16)
    bqf = np.asarray(bq, np.float32)
    bkf = np.asarray(bk, np.float32)
    bvf = np.asarray(bv, np.float32)

    qT, kT, vT = q.T, k.T, v.T  # [1024, 8192] views
    in_maps = []
    for c in range(8):
        a, sc, b, rc = c % 2, c // 2, c % 4, c // 4
        rows_g0 = slice(2048 * sc, 2048 * (sc + 1))
        rows_g1 = slice(4096 * rc + 1, 4096 * (rc + 1), 2)
        hrows = np.r_[256 * a:256 * a + 256, 512 + 128 * b:512 + 128 * b + 128]
        m = {
            "xa_q": np.ascontiguousarray(qT[:, rows_g0]),
            "xa_k": np.ascontiguousarray(kT[:, rows_g0]),
            "xa_v": np.ascontiguousarray(vT[:, rows_g0]),
            "xb_q": np.ascontiguousarray(qT[:, rows_g1]),
            "xb_k": np.ascontiguousarray(kT[:, rows_g1]),
            "xb_v": np.ascontiguousarray(vT[:, rows_g1]),
            "wq": np.ascontiguousarray(wq[hrows].T),
            "wk": np.ascontiguousarray(wk[hrows].T),
            "wv": np.ascontiguousarray(wv[hrows].T),
            "wo": np.ascontiguousarray(wo_f[:, hrows].T),
            "bq": np.ascontiguousarray(bqf[hrows]).reshape(384, 1),
            "bk": np.ascontiguousarray(bkf[hrows]).reshape(384, 1),
            "bv": np.ascontiguousarray(bvf[hrows]).reshape(384, 1),
        }
        in_maps.append(m)
    return in_maps


def _combine(results, bo):
    y = np.zeros((8192, 1024), np.float32)
    for c in range(8):
        sc, rc = c // 2, c // 4
        y[2048 * sc:2048 * (sc + 1)] += results[c]["y0"]
        y[4096 * rc + 1:4096 * (rc + 1):2] += results[c]["y1"]
    y += np.asarray(bo, np.float32)
    return y.reshape(1, 8192, 1024)


def kernel(query, key, value, Wq, bq, Wk, bk, Wv, bv, Wo, bo,
           _trace=False, _trace_cores=None):
    from concourse import bass_utils
    nc = _get_program()
    in_maps = _prep_inputs(query, key, value, Wq, bq, Wk, bk, Wv, bv, Wo, bo)
    res = bass_utils.run_bass_kernel_spmd(
        nc, in_maps, core_ids=list(range(8)),
        trace=_trace, trace_cores=_trace_cores)
    _CACHE["last_results"] = res
    return _combine(res.results, bo)



# revision 6
# speedup vs baseline: 1.0018x; 1.0018x over previous
"""BlockSparseRingMultiheadDilatedAttention Trainium2 kernel (v2).

Problem (hardcoded): B=1, N=8192, E=1024, H=16 heads, D=64.
Two dilated groups: g0 = heads 0-7, seg 2048, dilation 1;
                    g1 = heads 8-15, seg 4096, dilation 2, offset 1 (odd positions).
Causal within each (gathered) segment.

Sharding over 8 cores (uniform SPMD program, per-core data):
  core c: a = c%2, sc = c//2, b = c%4, rc = c//4
    g0: seg sc (rows 2048*sc .. +2048), heads 4a..4a+4   (4 blocks of [2048 x 2048])
    g1: seg rc odd rows (gathered, 2048 rows), heads 8+2b..+2 (2 blocks)
  Host pre-slices inputs (bf16 cast, odd-row gather, weight head slices) so the
  device program is identical on every core.  Host sums the per-core partial
  output projections (disjoint head contributions, bf16) and adds bo.

v2 device dataflow (vs v1 baseline):
  - attention processes BOTH heads of a pair per chunk: two row-tiled
    (tile_position (0,0)/(64,0)) K=64 S-matmuls share the PE array and land in
    one 2-bank PSUM tile -> ONE exp per [128,1024] covers both heads
  - softmax denominators via the appended-ones row as before, but
    normalization is restaged: numerator+denominator copied to SBUF (releases
    PSUM fast), reciprocal_approx_fast (~5x faster than InstReciprocal),
    gpsimd broadcast, one mul -> oT.  No multi-us reciprocal in the PE
    dependency chain (v1's HAM-oscillation cause).
  - pair-2 qkv projections + V transposes are emitted as fillers inside
    attention pair 0's ACT(exp)-limited windows; y0 out-proj chunks stream
    inside pair 1/2's windows; y1 inside pair 2 + small tail.
  - outputs in bf16 (host combines in fp32).
"""

import numpy as np
import ml_dtypes
from collections import deque

BF16 = ml_dtypes.bfloat16

SEG = 2048          # rows per attention block (both groups, post-gather)
E = 1024            # embedding
NQ = 512            # tq chunk (one PSUM bank of fp32)
NTQ = SEG // NQ     # 4 tq chunks per block
NTK = SEG // 128    # 16 tk chunks per block
ECH = E // 128      # 8 embedding chunks

_CACHE = {}


def _build_program():
    import concourse.bacc as bacc
    import concourse.mybir as mybir
    import concourse.tile as tile

    dt = mybir.dt
    nc = bacc.Bacc("TRN2", target_bir_lowering=False, debug=False,
                   enable_asserts=False)

    # ---- DRAM I/O (uniform across cores; host slices per core) ----
    xs = {}
    for sel in ("a", "b"):      # a = g0 rows, b = g1 gathered odd rows
        for inp in ("q", "k", "v"):
            xs[(sel, inp)] = nc.dram_tensor(
                f"x{sel}_{inp}", [E, SEG], dt.bfloat16, kind="ExternalInput").ap()
    ws = {inp: nc.dram_tensor(f"w{inp}", [E, 384], dt.bfloat16,
                              kind="ExternalInput").ap()
          for inp in ("q", "k", "v")}
    wo = nc.dram_tensor("wo", [384, E], dt.bfloat16, kind="ExternalInput").ap()
    bs = {inp: nc.dram_tensor(f"b{inp}", [384, 1], dt.float32,
                              kind="ExternalInput").ap()
          for inp in ("q", "k", "v")}
    y0 = nc.dram_tensor("y0", [SEG, E], dt.bfloat16, kind="ExternalOutput").ap()
    y1 = nc.dram_tensor("y1", [SEG, E], dt.bfloat16, kind="ExternalOutput").ap()

    with tile.TileContext(nc) as tc:
        from contextlib import ExitStack
        with ExitStack() as ctx:
            const = ctx.enter_context(tc.tile_pool(name="const", bufs=1))
            wpool = ctx.enter_context(tc.tile_pool(name="wpool", bufs=1))
            xpool = ctx.enter_context(tc.tile_pool(name="xpool", bufs=3))
            vtp = ctx.enter_context(tc.tile_pool(name="vtp", bufs=1))
            qkt = ctx.enter_context(tc.tile_pool(name="qkt", bufs=1))
            vnat = ctx.enter_context(tc.tile_pool(name="vnat", bufs=1))
            otp = ctx.enter_context(tc.tile_pool(name="otp", bufs=1))
            ptp = ctx.enter_context(tc.tile_pool(name="ptp", bufs=3))
            stg = ctx.enter_context(tc.tile_pool(name="stg", bufs=2))
            rcpp = ctx.enter_context(tc.tile_pool(name="rcpp", bufs=2))
            rbp = ctx.enter_context(tc.tile_pool(name="rbp", bufs=2))
            ysb = ctx.enter_context(tc.tile_pool(name="ysb", bufs=2))
            ps_s = ctx.enter_context(
                tc.tile_pool(name="ps_s", bufs=2, space="PSUM"))
            ps_b = ctx.enter_context(
                tc.tile_pool(name="ps_b", bufs=1, space="PSUM"))
            ps_o = ctx.enter_context(
                tc.tile_pool(name="ps_o", bufs=2, space="PSUM"))

            # ---- constants: identity (PE transpose), causal chunk masks ----
            ident = const.tile([128, 128], dt.bfloat16, tag="ident")
            nc.gpsimd.memset(ident, 1.0)
            nc.gpsimd.affine_select(
                out=ident, in_=ident, compare_op=mybir.AluOpType.is_equal,
                fill=0.0, base=0, pattern=[[-1, 128]], channel_multiplier=1)
            # mask[di] ([128,1024], both 512-halves identical): for k-chunk
            # i = 4j + di vs q-chunk j: keep where f - p - 128*di >= 0
            masks = []
            for di in range(4):
                m = const.tile([128, 2 * NQ], dt.bfloat16, tag=f"mask{di}")
                nc.gpsimd.memset(m, 1.0)
                nc.gpsimd.affine_select(
                    out=m.rearrange("p (hh f) -> p hh f", hh=2),
                    in_=m.rearrange("p (hh f) -> p hh f", hh=2),
                    compare_op=mybir.AluOpType.is_ge,
                    fill=0.0, base=-128 * di, pattern=[[0, 2], [1, NQ]],
                    channel_multiplier=-1)
                masks.append(m)

            # ---- weights (host-pre-transposed; plain DMA loads) ----
            wTa = {}
            for inp in ("q", "k", "v"):
                t = wpool.tile([128, 384 * ECH], dt.bfloat16,
                               tag=f"wT_{inp}", name=f"wT_{inp}")
                for ec in range(ECH):
                    nc.sync.dma_start(
                        out=t[:, 384 * ec:384 * (ec + 1)],
                        in_=ws[inp][128 * ec:128 * (ec + 1), :])
                wTa[inp] = t
            wT = {inp: [wTa[inp].rearrange("p (ec x) -> p ec x", x=384)
                        [:, :, 128 * p:128 * (p + 1)]
                        for p in range(3)] for inp in ("q", "k", "v")}
            woT = []
            for p in range(3):
                t = wpool.tile([128, E], dt.bfloat16, tag=f"woT_{p}")
                nc.sync.dma_start(out=t, in_=wo[128 * p:128 * (p + 1), :])
                woT.append(t)
            bsb = {}
            for inp in ("q", "k", "v"):
                for p in range(3):
                    t = wpool.tile([128, 1], dt.float32, tag=f"b_{inp}_{p}")
                    nc.sync.dma_start(
                        out=t, in_=bs[inp][128 * p:128 * (p + 1), :])
                    bsb[(inp, p)] = t

            # ---- persistent per-pair activations ----
            qT = [qkt.tile([128, SEG], dt.bfloat16, tag=f"qT{p}", name=f"qT{p}")
                  for p in range(3)]
            kT = [qkt.tile([128, SEG], dt.bfloat16, tag=f"kT{p}", name=f"kT{p}")
                  for p in range(3)]
            # V natural + ones col: per pair [128, 16*130]; chunk i at 130*i,
            # head h lhsT = [:, 130*i + 65*h : +65]
            vn = [vnat.tile([128, NTK * 130], dt.bfloat16, tag=f"vn{p}",
                            name=f"vn{p}")
                  for p in range(3)]
            for p in range(3):
                ones_view = vn[p].rearrange("p (k x) -> p k x", x=130)
                nc.gpsimd.memset(ones_view[:, :, 64:65], 1.0)
                nc.gpsimd.memset(ones_view[:, :, 129:130], 1.0)
            oT = [otp.tile([128, SEG], dt.bfloat16, tag=f"oT{p}", name=f"oT{p}")
                  for p in range(3)]

            # ---- input loads (all emitted up front; pool ring + DMA queue
            # FIFO sequence them: xa_q, xa_k, xa_v, xb_q, xb_k, xb_v) ----
            xt_tiles = {}
            for sel, inp in (("a", "q"), ("a", "k"), ("a", "v"),
                             ("b", "q"), ("b", "k"), ("b", "v")):
                t = xpool.tile([128, ECH * SEG], dt.bfloat16, tag="xt")
                for ec in range(ECH):
                    nc.sync.dma_start(
                        out=t[:, SEG * ec:SEG * (ec + 1)],
                        in_=xs[(sel, inp)][128 * ec:128 * (ec + 1), :])
                xt_tiles[(sel, inp)] = t

            # ---- projection building blocks ----
            def proj_half(acc, inp, p, t2, half, sel):
                xt = xt_tiles[(sel, inp)]
                c0 = NQ * (2 * t2 + half)
                for ec in range(ECH):
                    nc.tensor.matmul(
                        acc[:, NQ * half:NQ * (half + 1)],
                        wT[inp][p][:, ec, :],
                        xt[:, SEG * ec + c0:SEG * ec + c0 + NQ],
                        start=(ec == 0), stop=(ec == ECH - 1))

            def vtrans_batch(p, i0, vtile):
                # transpose 4 [128,128] chunks of vT into V-natural slices
                ptr = ps_b.tile([128, 4 * 128], dt.bfloat16, tag="b",
                                name="ptr")
                for qq in range(4):
                    nc.tensor.transpose(
                        ptr[:, 128 * qq:128 * (qq + 1)],
                        vtile[:, 128 * (i0 + qq):128 * (i0 + qq + 1)], ident)
                src = ptr[:, 0:512].rearrange("p (c h d) -> p c h d", c=4, h=2)
                dst = vn[p][:, 130 * i0:130 * (i0 + 4)].rearrange(
                    "p (c h x) -> p c h x", c=4, x=65)[:, :, :, 0:64]
                nc.vector.tensor_copy(dst, src)

            # ---- Phase A: projections for pairs 0,1 (PE-dense, DMA-paced) --
            for inp in ("q", "k", "v"):
                for p in (0, 1):
                    if inp == "v":
                        dst = vtp.tile([128, SEG], dt.bfloat16, tag="vT")
                    else:
                        dst = (qT if inp == "q" else kT)[p]
                    for t2 in range(2):
                        acc = ps_s.tile([128, 2 * NQ], dt.float32, tag="s",
                                        name="projA")
                        proj_half(acc, inp, p, t2, 0, "a")
                        proj_half(acc, inp, p, t2, 1, "a")
                        nc.scalar.activation(
                            dst[:, 2 * NQ * t2:2 * NQ * (t2 + 1)], acc,
                            mybir.ActivationFunctionType.Identity,
                            bias=bsb[(inp, p)], scale=1.0)
                    if inp == "v":
                        for i0 in (0, 4, 8, 12):
                            vtrans_batch(p, i0, dst)

            # ---- filler machinery ----
            fillers = deque()   # (min_slot, thunk)
            slot_counter = [0]

            def slot_cb():
                slot_counter[0] += 1
                if fillers and fillers[0][0] <= slot_counter[0]:
                    fillers.popleft()[1]()

            def drain_fillers():
                while fillers:
                    fillers.popleft()[1]()

            # pair-2 projection fillers (consumed inside attention pair 0).
            # xb DMA arrival (FIFO behind xa): q ~slot 4, k ~12, v ~22.
            def make_projB_fillers():
                cell = {}

                def unitA(inp, p, t2, dst):
                    def th():
                        acc = ps_b.tile([128, 2 * NQ], dt.float32, tag="b",
                                        name="projB")
                        cell[(inp, t2)] = acc
                        proj_half(acc, inp, p, t2, 0, "b")
                    return th

                def unitB(inp, p, t2, dst):
                    def th():
                        acc = cell.pop((inp, t2))
                        proj_half(acc, inp, p, t2, 1, "b")
                        nc.vector.tensor_scalar_add(
                            dst[:, 2 * NQ * t2:2 * NQ * (t2 + 1)], acc,
                            bsb[(inp, p)])
                    return th

                vtile = vtp.tile([128, SEG], dt.bfloat16, tag="vT")
                gates = {"q": 5, "k": 13, "v": 23}
                for inp in ("q", "k", "v"):
                    dst = {"q": qT[2], "k": kT[2], "v": vtile}[inp]
                    g = gates[inp]
                    for t2 in range(2):
                        fillers.append((g + 2 * t2, unitA(inp, 2, t2, dst)))
                        fillers.append((g + 2 * t2 + 1, unitB(inp, 2, t2, dst)))
                for bi, i0 in enumerate((0, 4, 8, 12)):
                    fillers.append(
                        (28 + 2 * bi,
                         (lambda i0=i0: vtrans_batch(2, i0, vtile))))

            # out-projection chunk fillers
            def y_unit(ydram, pairs, m):
                def th():
                    accy = ps_b.tile([128, 2 * NQ], dt.float32, tag="b",
                                     name="accy")
                    for jc in range(2):
                        for idx, p in enumerate(pairs):
                            nc.tensor.matmul(
                                accy[:, NQ * jc:NQ * (jc + 1)],
                                oT[p][:, 128 * m:128 * (m + 1)],
                                woT[p][:, NQ * jc:NQ * (jc + 1)],
                                start=(idx == 0), stop=(idx == len(pairs) - 1))
                    t = ysb.tile([128, 2 * NQ], dt.bfloat16, tag="ysb")
                    nc.vector.tensor_copy(t, accy)
                    nc.sync.dma_start(
                        out=ydram[128 * m:128 * (m + 1), :], in_=t)
                return th

            # ---- Phase B: attention (both heads of a pair per chunk) ----
            def norm_emit(p, j, acc0, acc1):
                for h, acc in enumerate((acc0, acc1)):
                    st = stg.tile([65, NQ], dt.float32, tag="stg")
                    nc.vector.tensor_copy(st, acc[0:65, :])
                    rc = rcpp.tile([1, NQ], dt.float32, tag="rcp")
                    nc.vector.reciprocal(rc, st[64:65, :])
                    rb = rbp.tile([64, NQ], dt.float32, tag="rb")
                    nc.gpsimd.partition_broadcast(rb, rc)
                    nc.vector.tensor_mul(
                        oT[p][64 * h:64 * h + 64, NQ * j:NQ * (j + 1)],
                        st[0:64, :], rb)

            def attn_pair(p, after_norm=None):
                for j in range(NTQ):
                    nchunks = 4 * (j + 1)
                    acc0 = ps_o.tile([128, NQ], dt.float32, tag="acc",
                                     name="acc0")
                    acc1 = ps_o.tile([128, NQ], dt.float32, tag="acc",
                                     name="acc1")

                    def s_emit(ci):
                        s = ps_s.tile([128, 2 * NQ], dt.float32, tag="s",
                                      name="s")
                        nc.tensor.matmul(
                            s[:, 0:NQ],
                            kT[p][0:64, 128 * ci:128 * (ci + 1)],
                            qT[p][0:64, NQ * j:NQ * (j + 1)],
                            start=True, stop=True)
                        nc.tensor.matmul(
                            s[:, NQ:2 * NQ],
                            kT[p][64:128, 128 * ci:128 * (ci + 1)],
                            qT[p][64:128, NQ * j:NQ * (j + 1)],
                            start=True, stop=True)
                        return s

                    s = s_emit(0)
                    for ci in range(nchunks):
                        pt = ptp.tile([128, 2 * NQ], dt.bfloat16, tag="pt",
                                      name="pt")
                        nc.scalar.activation(
                            pt, s, mybir.ActivationFunctionType.Exp,
                            bias=0.0, scale=0.125)
                        di = ci - 4 * j
                        if di >= 0:
                            nc.vector.tensor_mul(pt, pt, masks[di])
                        if ci + 1 < nchunks:
                            s = s_emit(ci + 1)
                        last = (ci == nchunks - 1)
                        nc.tensor.matmul(
                            acc0[0:65, :],
                            vn[p][:, 130 * ci:130 * ci + 65],
                            pt[:, 0:NQ], start=(ci == 0), stop=last)
                        nc.tensor.matmul(
                            acc1[0:65, :],
                            vn[p][:, 130 * ci + 65:130 * ci + 130],
                            pt[:, NQ:2 * NQ], start=(ci == 0), stop=last)
                        slot_cb()
                    norm_emit(p, j, acc0, acc1)
                    if after_norm is not None:
                        after_norm(j)

            make_projB_fillers()
            attn_pair(0)
            drain_fillers()
            slot_counter[0] = 0

            def queue_y0(j):
                for m in range(4 * j, 4 * j + 4):
                    fillers.append((0, y_unit(y0, (0, 1), m)))
            attn_pair(1, after_norm=queue_y0)
            slot_counter[0] = 0

            def queue_y1(j):
                for m in range(4 * j, 4 * j + 4):
                    fillers.append((0, y_unit(y1, (2,), m)))
            attn_pair(2, after_norm=queue_y1)
            drain_fillers()

    nc.compile()
    return nc


def _get_program():
    if "nc" not in _CACHE:
        _CACHE["nc"] = _build_program()
    return _CACHE["nc"]


def _prep_inputs(query, key, value, Wq, bq, Wk, bk, Wv, bv, Wo, bo):
    """Build the 8 per-core input maps (host-side slicing + bf16 cast)."""
    q = np.asarray(query, np.float32).reshape(8192, 1024).astype(BF16)
    k = np.asarray(key, np.float32).reshape(8192, 1024).astype(BF16)
    v = np.asarray(value, np.float32).reshape(8192, 1024).astype(BF16)
    wq = np.asarray(Wq, np.float32).astype(BF16)
    wk = np.asarray(Wk, np.float32).astype(BF16)
    wv = np.asarray(Wv, np.float32).astype(BF16)
    wo_f = np.asarray(Wo, np.float32).astype(BF16)
    bqf = np.asarray(bq, np.float32)
    bkf = np.asarray(bk, np.float32)
    bvf = np.asarray(bv, np.float32)

    qT, kT, vT = q.T, k.T, v.T  # [1024, 8192] views
    in_maps = []
    for c in range(8):
        a, sc, b, rc = c % 2, c // 2, c % 4, c // 4
        rows_g0 = slice(2048 * sc, 2048 * (sc + 1))
        rows_g1 = slice(4096 * rc + 1, 4096 * (rc + 1), 2)
        hrows = np.r_[256 * a:256 * a + 256, 512 + 128 * b:512 + 128 * b + 128]
        m = {
            "xa_q": np.ascontiguousarray(qT[:, rows_g0]),
            "xa_k": np.ascontiguousarray(kT[:, rows_g0]),
            "xa_v": np.ascontiguousarray(vT[:, rows_g0]),
            "xb_q": np.ascontiguousarray(qT[:, rows_g1]),
            "xb_k": np.ascontiguousarray(kT[:, rows_g1]),
            "xb_v": np.ascontiguousarray(vT[:, rows_g1]),
            "wq": np.ascontiguousarray(wq[hrows].T),
            "wk": np.ascontiguousarray(wk[hrows].T),
            "wv": np.ascontiguousarray(wv[hrows].T),
            "wo": np.ascontiguousarray(wo_f[:, hrows].T),
            "bq": np.ascontiguousarray(bqf[hrows]).reshape(384, 1),
            "bk": np.ascontiguousarray(bkf[hrows]).reshape(384, 1),
            "bv": np.ascontiguousarray(bvf[hrows]).reshape(384, 1),
        }
        in_maps.append(m)
    return in_maps


def _combine(results, bo):
    y = np.zeros((8192, 1024), np.float32)
    for c in range(8):
        sc, rc = c // 2, c // 4
        y[2048 * sc:2048 * (sc + 1)] += results[c]["y0"].astype(np.float32)
        y[4096 * rc + 1:4096 * (rc + 1):2] += results[c]["y1"].astype(np.float32)
    y += np.asarray(bo, np.float32)
    return y.reshape(1, 8192, 1024)


def kernel(query, key, value, Wq, bq, Wk, bk, Wv, bv, Wo, bo,
           _trace=False, _trace_cores=None):
    from concourse import bass_utils
    nc = _get_program()
    in_maps = _prep_inputs(query, key, value, Wq, bq, Wk, bk, Wv, bv, Wo, bo)
    res = bass_utils.run_bass_kernel_spmd(
        nc, in_maps, core_ids=list(range(8)),
        trace=_trace, trace_cores=_trace_cores)
    _CACHE["last_results"] = res
    return _combine(res.results, bo)


# revision 12
# speedup vs baseline: 1.2055x; 1.2033x over previous
"""BlockSparseRingMultiheadDilatedAttention Trainium2 kernel (v3).

Problem (hardcoded): B=1, N=8192, E=1024, H=16 heads, D=64.
Two dilated groups: g0 = heads 0-7, seg 2048, dilation 1;
                    g1 = heads 8-15, seg 4096, dilation 2, offset 1 (odd positions).
Causal within each (gathered) segment.

Sharding over 8 cores (uniform SPMD program, per-core data):
  core c: a = c%2, sc = c//2, b = c%4, rc = c//4
    g0: seg sc (rows 2048*sc .. +2048), heads 4a..4a+4   (4 blocks of [2048 x 2048])
    g1: seg rc odd rows (gathered, 2048 rows), heads 8+2b..+2 (2 blocks)
  Host pre-slices inputs (bf16 cast, odd-row gather, weight head slices,
  SBUF-layout rearrange) so the device program is identical on every core.
  Host sums the per-core partial output projections (disjoint head
  contributions, bf16) and adds bo.

v3 device dataflow:
  - few LARGE input DMAs (2 per x tensor, 1 per weight tensor) in final SBUF
    layout -- per-DMA fixed costs gated startup in v1/v2 (single HW queue,
    ~2us/DMA completion latency)
  - attention processes BOTH heads of a pair per chunk: two row-tiled
    (tile_position (0,0)/(64,0)) K=64 S-matmuls run concurrently in the PE
    array, land in one 2-bank PSUM tile -> ONE exp per [128,1024]
  - softmax: denominators from the appended-ones row of the PV matmul;
    numerators copied UNNORMALIZED to oT (releases PSUM fast); denominator
    rows gathered onto 8 partitions of one tile -> ONE batched DVE
    reciprocal per pair; broadcast+normalize muls run as fillers inside the
    NEXT pair's window (keeps the DVE FIFO from stalling PE -> HAM warm)
  - pair-2 qkv projections + V transposes are fillers inside pair 0/1's
    ACT(exp)-limited windows; y0/y1 out-proj chunks stream inside pair 2
  - outputs in bf16 (host combines in fp32)
"""

import numpy as np
import ml_dtypes
from collections import deque

BF16 = ml_dtypes.bfloat16

SEG = 2048          # rows per attention block (both groups, post-gather)
E = 1024            # embedding
NQ = 512            # tq chunk (one PSUM bank of fp32)
NTQ = SEG // NQ     # 4 tq chunks per block
NTK = SEG // 128    # 16 tk chunks per block
ECH = E // 128      # 8 embedding chunks

_CACHE = {}


def _build_program():
    import concourse.bacc as bacc
    import concourse.mybir as mybir
    import concourse.tile as tile

    dt = mybir.dt
    nc = bacc.Bacc("TRN2", target_bir_lowering=False, debug=False,
                   enable_asserts=False)

    # ---- DRAM I/O (uniform across cores; host slices per core) ----
    # x tensors already in SBUF layout [128, 2(pos half) * 8(ec) * 1024(pos)]
    xs = {}
    for sel in ("a", "b"):
        for inp in ("q", "k", "v"):
            xs[(sel, inp)] = nc.dram_tensor(
                f"x{sel}_{inp}", [128, 2 * ECH * 1024], dt.bfloat16,
                kind="ExternalInput").ap()
    ws = {inp: nc.dram_tensor(f"w{inp}", [128, 384 * ECH], dt.bfloat16,
                              kind="ExternalInput").ap()
          for inp in ("q", "k", "v")}
    wo = nc.dram_tensor("wo", [128, 3 * E], dt.bfloat16,
                        kind="ExternalInput").ap()
    ball = nc.dram_tensor("ball", [128, 9], dt.float32,
                          kind="ExternalInput").ap()
    y0 = nc.dram_tensor("y0", [SEG, E], dt.bfloat16, kind="ExternalOutput").ap()
    y1 = nc.dram_tensor("y1", [SEG, E], dt.bfloat16, kind="ExternalOutput").ap()

    with tile.TileContext(nc) as tc:
        from contextlib import ExitStack
        with ExitStack() as ctx:
            const = ctx.enter_context(tc.tile_pool(name="const", bufs=1))
            wpool = ctx.enter_context(tc.tile_pool(name="wpool", bufs=1))
            xpool = ctx.enter_context(tc.tile_pool(name="xpool", bufs=3))
            vtp = ctx.enter_context(tc.tile_pool(name="vtp", bufs=1))
            qkt = ctx.enter_context(tc.tile_pool(name="qkt", bufs=1))
            vnat = ctx.enter_context(tc.tile_pool(name="vnat", bufs=1))
            otp = ctx.enter_context(tc.tile_pool(name="otp", bufs=1))
            ptp = ctx.enter_context(tc.tile_pool(name="ptp", bufs=2))
            dnp = ctx.enter_context(tc.tile_pool(name="dnp", bufs=1))
            dsp_p = ctx.enter_context(tc.tile_pool(name="dsp_p", bufs=1))
            rcq = ctx.enter_context(tc.tile_pool(name="rcq", bufs=2))
            rbp = ctx.enter_context(tc.tile_pool(name="rbp", bufs=2))
            ysb = ctx.enter_context(tc.tile_pool(name="ysb", bufs=2))
            ps_s = ctx.enter_context(
                tc.tile_pool(name="ps_s", bufs=2, space="PSUM"))
            ps_b = ctx.enter_context(
                tc.tile_pool(name="ps_b", bufs=1, space="PSUM"))
            ps_o = ctx.enter_context(
                tc.tile_pool(name="ps_o", bufs=2, space="PSUM"))

            # ---- constants: identity (PE transpose), causal chunk masks ----
            ident = const.tile([128, 128], dt.bfloat16, tag="ident")
            nc.gpsimd.memset(ident, 1.0)
            nc.gpsimd.affine_select(
                out=ident, in_=ident, compare_op=mybir.AluOpType.is_equal,
                fill=0.0, base=0, pattern=[[-1, 128]], channel_multiplier=1)
            # mask[di] ([128,1024], both 512-halves identical): for k-chunk
            # i = 4j + di vs q-chunk j: keep where f - p - 128*di >= 0
            masks = []
            for di in range(4):
                m = const.tile([128, 2 * NQ], dt.bfloat16, tag=f"mask{di}")
                nc.gpsimd.memset(m, 1.0)
                nc.gpsimd.affine_select(
                    out=m.rearrange("p (hh f) -> p hh f", hh=2),
                    in_=m.rearrange("p (hh f) -> p hh f", hh=2),
                    compare_op=mybir.AluOpType.is_ge,
                    fill=0.0, base=-128 * di, pattern=[[0, 2], [1, NQ]],
                    channel_multiplier=-1)
                masks.append(m)

            # ---- weights: one DMA per tensor (host pre-layouts) ----
            wTa = {}
            for inp in ("q", "k", "v"):
                t = wpool.tile([128, 384 * ECH], dt.bfloat16,
                               tag=f"wT_{inp}", name=f"wT_{inp}")
                nc.sync.dma_start(out=t, in_=ws[inp])
                wTa[inp] = t
            wT = {inp: [wTa[inp].rearrange("p (ec x) -> p ec x", x=384)
                        [:, :, 128 * p:128 * (p + 1)]
                        for p in range(3)] for inp in ("q", "k", "v")}
            woTa = wpool.tile([128, 3 * E], dt.bfloat16, tag="woT")
            nc.sync.dma_start(out=woTa, in_=wo)
            woT = [woTa[:, E * p:E * (p + 1)] for p in range(3)]
            ballt = wpool.tile([128, 9], dt.float32, tag="ball")
            nc.sync.dma_start(out=ballt, in_=ball)
            bsb = {}
            for i, inp in enumerate(("q", "k", "v")):
                for p in range(3):
                    bsb[(inp, p)] = ballt[:, 3 * i + p:3 * i + p + 1]

            # ---- persistent per-pair activations ----
            qT = [qkt.tile([128, SEG], dt.bfloat16, tag=f"qT{p}", name=f"qT{p}")
                  for p in range(3)]
            kT = [qkt.tile([128, SEG], dt.bfloat16, tag=f"kT{p}", name=f"kT{p}")
                  for p in range(3)]
            vn = [vnat.tile([128, NTK * 130], dt.bfloat16, tag=f"vn{p}",
                            name=f"vn{p}")
                  for p in range(3)]
            for p in range(3):
                ones_view = vn[p].rearrange("p (k x) -> p k x", x=130)
                nc.gpsimd.memset(ones_view[:, :, 64:65], 1.0)
                nc.gpsimd.memset(ones_view[:, :, 129:130], 1.0)
            oT = [otp.tile([128, SEG], dt.bfloat16, tag=f"oT{p}", name=f"oT{p}")
                  for p in range(3)]

            # ---- input loads: 2 DMAs (position halves) per tensor ----
            xt_tiles = {}
            HC = ECH * 1024     # columns per position half
            for sel, inp in (("a", "q"), ("a", "k"), ("a", "v"),
                             ("b", "q"), ("b", "k"), ("b", "v")):
                t = xpool.tile([128, 2 * HC], dt.bfloat16, tag="xt")
                for t2 in range(2):
                    nc.sync.dma_start(
                        out=t[:, HC * t2:HC * (t2 + 1)],
                        in_=xs[(sel, inp)][:, HC * t2:HC * (t2 + 1)])
                # view [p, t2, ec, pos]
                xt_tiles[(sel, inp)] = t.rearrange(
                    "p (t2 ec s) -> p t2 ec s", t2=2, ec=ECH)

            # ---- projection building blocks ----
            def proj_half(acc, inp, p, t2, half, sel):
                xt = xt_tiles[(sel, inp)]
                for ec in range(ECH):
                    nc.tensor.matmul(
                        acc[:, NQ * half:NQ * (half + 1)],
                        wT[inp][p][:, ec, :],
                        xt[:, t2, ec, NQ * half:NQ * (half + 1)],
                        start=(ec == 0), stop=(ec == ECH - 1))

            def vtrans_batch(p, i0, vtile):
                # transpose 4 [128,128] chunks of vT into V-natural slices
                ptr = ps_b.tile([128, 4 * 128], dt.bfloat16, tag="b",
                                name="ptr")
                for qq in range(4):
                    nc.tensor.transpose(
                        ptr[:, 128 * qq:128 * (qq + 1)],
                        vtile[:, 128 * (i0 + qq):128 * (i0 + qq + 1)], ident)
                src = ptr.rearrange("p (c h d) -> p c h d", c=4, h=2)
                dst = vn[p][:, 130 * i0:130 * (i0 + 4)].rearrange(
                    "p (c h x) -> p c h x", c=4, x=65)[:, :, :, 0:64]
                nc.vector.tensor_copy(dst, src)

            # ---- Phase A: projections for pairs 0,1 (PE-dense, DMA-paced) --
            for inp in ("q", "k", "v"):
                for p in (0, 1):
                    if inp == "v":
                        dst = vtp.tile([128, SEG], dt.bfloat16, tag="vT")
                    else:
                        dst = (qT if inp == "q" else kT)[p]
                    for t2 in range(2):
                        acc = ps_s.tile([128, 2 * NQ], dt.float32, tag="s",
                                        name="projA")
                        proj_half(acc, inp, p, t2, 0, "a")
                        proj_half(acc, inp, p, t2, 1, "a")
                        nc.scalar.activation(
                            dst[:, 2 * NQ * t2:2 * NQ * (t2 + 1)], acc,
                            mybir.ActivationFunctionType.Identity,
                            bias=bsb[(inp, p)], scale=1.0)
                    if inp == "v":
                        for i0 in (0, 4, 8, 12):
                            vtrans_batch(p, i0, dst)

            # ---- filler machinery ----
            fillers = deque()   # (min_slot, thunk)
            slot_counter = [0]

            def slot_cb():
                slot_counter[0] += 1
                if fillers and fillers[0][0] <= slot_counter[0]:
                    fillers.popleft()[1]()

            def drain_fillers():
                while fillers:
                    fillers.popleft()[1]()

            # pair-2 projection fillers (consumed inside attention pair 0).
            def make_projB_fillers():
                cell = {}

                def unitA(inp, p, t2, dst):
                    def th():
                        acc = ps_b.tile([128, 2 * NQ], dt.float32, tag="b",
                                        name="projB")
                        cell[(inp, t2)] = acc
                        proj_half(acc, inp, p, t2, 0, "b")
                    return th

                def unitB(inp, p, t2, dst):
                    def th():
                        acc = cell.pop((inp, t2))
                        proj_half(acc, inp, p, t2, 1, "b")
                        nc.vector.tensor_scalar_add(
                            dst[:, 2 * NQ * t2:2 * NQ * (t2 + 1)], acc,
                            bsb[(inp, p)])
                    return th

                vtile = vtp.tile([128, SEG], dt.bfloat16, tag="vT")
                gates = {"q": 4, "k": 12, "v": 22}
                for inp in ("q", "k", "v"):
                    dst = {"q": qT[2], "k": kT[2], "v": vtile}[inp]
                    g = gates[inp]
                    for t2 in range(2):
                        fillers.append((g + 2 * t2, unitA(inp, 2, t2, dst)))
                        fillers.append((g + 2 * t2 + 1, unitB(inp, 2, t2, dst)))
                for bi, i0 in enumerate((0, 4, 8, 12)):
                    fillers.append(
                        (27 + 2 * bi,
                         (lambda i0=i0: vtrans_batch(2, i0, vtile))))

            # out-projection chunk fillers
            def y_unit(ydram, pairs, m):
                def th():
                    accy = ps_b.tile([128, 2 * NQ], dt.float32, tag="b",
                                     name="accy")
                    for jc in range(2):
                        for idx, p in enumerate(pairs):
                            nc.tensor.matmul(
                                accy[:, NQ * jc:NQ * (jc + 1)],
                                oT[p][:, 128 * m:128 * (m + 1)],
                                woT[p][:, NQ * jc:NQ * (jc + 1)],
                                start=(idx == 0), stop=(idx == len(pairs) - 1))
                    t = ysb.tile([128, 2 * NQ], dt.bfloat16, tag="ysb")
                    nc.vector.tensor_copy(t, accy)
                    nc.sync.dma_start(
                        out=ydram[128 * m:128 * (m + 1), :], in_=t)
                return th

            # ---- normalization (DMA-scatter batched reciprocal) ----
            # per (p,j,h): den row [1,NQ] copied from acc[64:65] into dens at
            # partition row 32*(2*(j%2)+h), free half j//2 (allowed starts).
            # per pair: 8 scatter DMAs -> dsp[128,32]; ONE reciprocal; 8
            # gather DMAs back -> rcpr (same sparse-row layout).  Normalize
            # muls run as fillers in the next pair's window.
            def drow(j, h):
                return 32 * (2 * (j % 2) + h), NQ * (j // 2)

            dens = [dnp.tile([128, 2 * NQ], dt.float32, tag="dens",
                             name=f"dens{p}") for p in range(3)]
            dsp = [dsp_p.tile([128, 32], dt.float32, tag=f"dsp{p}",
                              name=f"dsp{p}") for p in range(3)]
            rsp = [dsp_p.tile([128, 32], dt.float32, tag=f"rsp{p}",
                              name=f"rsp{p}") for p in range(3)]

            def recip_batch(p, us):
                # us = contiguous list of unit indices u = 2*j + h
                def th():
                    for u in us:
                        r, c = drow(u // 2, u % 2)
                        nc.sync.dma_start(
                            out=dsp[p][:, 4 * u:4 * (u + 1)],
                            in_=dens[p][r:r + 1, c:c + NQ])
                    u0, u1 = us[0], us[-1] + 1
                    nc.vector.reciprocal(rsp[p][:, 4 * u0:4 * u1],
                                         dsp[p][:, 4 * u0:4 * u1])
                return th

            def norm_bc_unit(p, j, rb2s):
                # gather 1/den rows to partition-0 ring tiles via DMA;
                # broadcast head0 into rb2 lower half directly (base 0);
                # head1 into a base-0 temp then SBUF->SBUF DMA into the
                # upper half (gpsimd broadcast cannot write base 64).
                def th():
                    rb2 = rbp.tile([128, NQ], dt.float32, tag="rb")
                    rb2s[j] = rb2
                    for h in range(2):
                        u = 2 * j + h
                        rc = rcq.tile([1, NQ], dt.float32, tag="rc")
                        nc.sync.dma_start(out=rc,
                                          in_=rsp[p][:, 4 * u:4 * (u + 1)])
                        if h == 0:
                            nc.gpsimd.partition_broadcast(rb2[0:64, :], rc)
                        else:
                            rbt = rbp.tile([64, NQ], dt.float32, tag="rbt")
                            nc.gpsimd.partition_broadcast(rbt, rc)
                            nc.sync.dma_start(out=rb2[64:128, :], in_=rbt)
                return th

            def norm_mul_unit(p, j, rb2s):
                def th():
                    sl = oT[p][:, NQ * j:NQ * (j + 1)]
                    nc.vector.tensor_mul(sl, sl, rb2s.pop(j))
                return th

            def norm_fillers(p, base, js):
                rb2s = {}
                ents = [(base, recip_batch(p, [u for j in js
                                               for u in (2 * j, 2 * j + 1)]))]
                for i, j in enumerate(js):
                    ents.append((base + 2 + i, norm_bc_unit(p, j, rb2s)))
                    ents.append((base + 4 + i, norm_mul_unit(p, j, rb2s)))
                ents.sort(key=lambda e: e[0])
                return ents

            def attn_pair(p, after_j=None):
                for j in range(NTQ):
                    nchunks = 4 * (j + 1)
                    acc0 = ps_o.tile([128, NQ], dt.float32, tag="acc",
                                     name="acc0")
                    acc1 = ps_o.tile([128, NQ], dt.float32, tag="acc",
                                     name="acc1")

                    def s_emit(ci):
                        s = ps_s.tile([128, 2 * NQ], dt.float32, tag="s",
                                      name="s")
                        nc.tensor.matmul(
                            s[:, 0:NQ],
                            kT[p][0:64, 128 * ci:128 * (ci + 1)],
                            qT[p][0:64, NQ * j:NQ * (j + 1)],
                            start=True, stop=True)
                        nc.tensor.matmul(
                            s[:, NQ:2 * NQ],
                            kT[p][64:128, 128 * ci:128 * (ci + 1)],
                            qT[p][64:128, NQ * j:NQ * (j + 1)],
                            start=True, stop=True)
                        return s

                    s = s_emit(0)
                    for ci in range(nchunks):
                        pt = ptp.tile([128, 2 * NQ], dt.bfloat16, tag="pt",
                                      name="pt")
                        nc.scalar.activation(
                            pt, s, mybir.ActivationFunctionType.Exp,
                            bias=0.0, scale=0.125)
                        di = ci - 4 * j
                        if di >= 0:
                            nc.vector.tensor_mul(pt, pt, masks[di])
                        if ci + 1 < nchunks:
                            s = s_emit(ci + 1)
                        last = (ci == nchunks - 1)
                        nc.tensor.matmul(
                            acc0[0:65, :],
                            vn[p][:, 130 * ci:130 * ci + 65],
                            pt[:, 0:NQ], start=(ci == 0), stop=last)
                        nc.tensor.matmul(
                            acc1[0:65, :],
                            vn[p][:, 130 * ci + 65:130 * ci + 130],
                            pt[:, NQ:2 * NQ], start=(ci == 0), stop=last)
                        slot_cb()
                    # stage numerators (unnormalized) + denominator rows
                    for h, acc in enumerate((acc0, acc1)):
                        nc.vector.tensor_copy(
                            oT[p][64 * h:64 * h + 64, NQ * j:NQ * (j + 1)],
                            acc[0:64, :])
                        r, c = drow(j, h)
                        nc.vector.tensor_copy(
                            dens[p][r:r + 1, c:c + NQ], acc[64:65, :])
                    if after_j is not None:
                        after_j(j)

            make_projB_fillers()

            # pair 0; its norm runs as fillers at the start of pair 1
            attn_pair(0)
            slot_counter[0] = 0
            fillers.extend(norm_fillers(0, 0, [0, 1, 2, 3]))

            # pair 1; its norm + y0 units run inside pair 2
            attn_pair(1)
            slot_counter[0] = 0
            fillers.extend(norm_fillers(1, 0, [0, 1, 2, 3]))
            for m in range(16):
                fillers.append((9 + m, y_unit(y0, (0, 1), m)))

            # pair 2: j0..j2 norms + y1 j0..j2 units inside j=3's window
            def after_j2(j):
                if j == 2:
                    fillers.extend(norm_fillers(2, 25, [0, 1, 2]))
                    for m in range(12):
                        fillers.append((33 + m, y_unit(y1, (2,), m)))
            attn_pair(2, after_j=after_j2)
            drain_fillers()
            # tail: j3 norm + last y1 units
            rb2s_t = {}
            recip_batch(2, [6, 7])()
            norm_bc_unit(2, 3, rb2s_t)()
            norm_mul_unit(2, 3, rb2s_t)()
            for m in range(12, 16):
                y_unit(y1, (2,), m)()

    nc.compile()
    return nc


def _get_program():
    if "nc" not in _CACHE:
        _CACHE["nc"] = _build_program()
    return _CACHE["nc"]


def _sbuf_layout(xT):
    """[1024, 2048] (e, pos) -> [128, 2*8*1024]: pos-half major, ec, pos."""
    # [ec, 128, t2, 1024] -> [128, t2, ec, 1024]
    return np.ascontiguousarray(
        xT.reshape(ECH, 128, 2, 1024).transpose(1, 2, 0, 3).reshape(
            128, 2 * ECH * 1024))


def _prep_inputs(query, key, value, Wq, bq, Wk, bk, Wv, bv, Wo, bo):
    """Build the 8 per-core input maps (host-side slicing + bf16 cast)."""
    q = np.asarray(query, np.float32).reshape(8192, 1024).astype(BF16)
    k = np.asarray(key, np.float32).reshape(8192, 1024).astype(BF16)
    v = np.asarray(value, np.float32).reshape(8192, 1024).astype(BF16)
    wq = np.asarray(Wq, np.float32).astype(BF16)
    wk = np.asarray(Wk, np.float32).astype(BF16)
    wv = np.asarray(Wv, np.float32).astype(BF16)
    wo_f = np.asarray(Wo, np.float32).astype(BF16)
    bqf = np.asarray(bq, np.float32)
    bkf = np.asarray(bk, np.float32)
    bvf = np.asarray(bv, np.float32)

    qT, kT, vT = q.T, k.T, v.T  # [1024, 8192] views
    in_maps = []
    for c in range(8):
        a, sc, b, rc = c % 2, c // 2, c % 4, c // 4
        rows_g0 = slice(2048 * sc, 2048 * (sc + 1))
        rows_g1 = slice(4096 * rc + 1, 4096 * (rc + 1), 2)
        hrows = np.r_[256 * a:256 * a + 256, 512 + 128 * b:512 + 128 * b + 128]

        def wlay(w):
            # [1024, 384] -> [128, 8*384] (ec-blocked)
            return np.ascontiguousarray(
                w.reshape(ECH, 128, 384).transpose(1, 0, 2).reshape(128, -1))

        wov = np.ascontiguousarray(wo_f[:, hrows].T)  # [384, 1024]
        ballv = np.stack([np.ascontiguousarray(bf[hrows]).reshape(3, 128)
                          for bf in (bqf, bkf, bvf)], axis=0)  # [3,3,128]
        # ball[128, 9]: col 3i+p = input i, pair p
        ballv = np.ascontiguousarray(ballv.reshape(9, 128).T)
        m = {
            "xa_q": _sbuf_layout(np.ascontiguousarray(qT[:, rows_g0])),
            "xa_k": _sbuf_layout(np.ascontiguousarray(kT[:, rows_g0])),
            "xa_v": _sbuf_layout(np.ascontiguousarray(vT[:, rows_g0])),
            "xb_q": _sbuf_layout(np.ascontiguousarray(qT[:, rows_g1])),
            "xb_k": _sbuf_layout(np.ascontiguousarray(kT[:, rows_g1])),
            "xb_v": _sbuf_layout(np.ascontiguousarray(vT[:, rows_g1])),
            "wq": wlay(np.ascontiguousarray(wq[hrows].T)),
            "wk": wlay(np.ascontiguousarray(wk[hrows].T)),
            "wv": wlay(np.ascontiguousarray(wv[hrows].T)),
            "wo": np.ascontiguousarray(
                wov.reshape(3, 128, 1024).transpose(1, 0, 2).reshape(128, -1)),
            "ball": ballv,
        }
        in_maps.append(m)
    return in_maps


def _combine(results, bo):
    y = np.zeros((8192, 1024), np.float32)
    for c in range(8):
        sc, rc = c // 2, c // 4
        y[2048 * sc:2048 * (sc + 1)] += results[c]["y0"].astype(np.float32)
        y[4096 * rc + 1:4096 * (rc + 1):2] += results[c]["y1"].astype(np.float32)
    y += np.asarray(bo, np.float32)
    return y.reshape(1, 8192, 1024)


def kernel(query, key, value, Wq, bq, Wk, bk, Wv, bv, Wo, bo,
           _trace=False, _trace_cores=None):
    from concourse import bass_utils
    nc = _get_program()
    in_maps = _prep_inputs(query, key, value, Wq, bq, Wk, bk, Wv, bv, Wo, bo)
    res = bass_utils.run_bass_kernel_spmd(
        nc, in_maps, core_ids=list(range(8)),
        trace=_trace, trace_cores=_trace_cores)
    _CACHE["last_results"] = res
    return _combine(res.results, bo)


# revision 18
# speedup vs baseline: 1.2255x; 1.0166x over previous
"""BlockSparseRingMultiheadDilatedAttention Trainium2 kernel (v3).

Problem (hardcoded): B=1, N=8192, E=1024, H=16 heads, D=64.
Two dilated groups: g0 = heads 0-7, seg 2048, dilation 1;
                    g1 = heads 8-15, seg 4096, dilation 2, offset 1 (odd positions).
Causal within each (gathered) segment.

Sharding over 8 cores (uniform SPMD program, per-core data):
  core c: a = c%2, sc = c//2, b = c%4, rc = c//4
    g0: seg sc (rows 2048*sc .. +2048), heads 4a..4a+4   (4 blocks of [2048 x 2048])
    g1: seg rc odd rows (gathered, 2048 rows), heads 8+2b..+2 (2 blocks)
  Host pre-slices inputs (bf16 cast, odd-row gather, weight head slices,
  SBUF-layout rearrange) so the device program is identical on every core.
  Host sums the per-core partial output projections (disjoint head
  contributions, bf16) and adds bo.

v3 device dataflow:
  - few LARGE input DMAs (2 per x tensor, 1 per weight tensor) in final SBUF
    layout -- per-DMA fixed costs gated startup in v1/v2 (single HW queue,
    ~2us/DMA completion latency)
  - attention processes BOTH heads of a pair per chunk: two row-tiled
    (tile_position (0,0)/(64,0)) K=64 S-matmuls run concurrently in the PE
    array, land in one 2-bank PSUM tile -> ONE exp per [128,1024]
  - softmax: denominators from the appended-ones row of the PV matmul;
    numerators copied UNNORMALIZED to oT (releases PSUM fast); denominator
    rows gathered onto 8 partitions of one tile -> ONE batched DVE
    reciprocal per pair; broadcast+normalize muls run as fillers inside the
    NEXT pair's window (keeps the DVE FIFO from stalling PE -> HAM warm)
  - pair-2 qkv projections + V transposes are fillers inside pair 0/1's
    ACT(exp)-limited windows; y0/y1 out-proj chunks stream inside pair 2
  - outputs in bf16 (host combines in fp32)
"""

import numpy as np
import ml_dtypes
from collections import deque

BF16 = ml_dtypes.bfloat16

SEG = 2048          # rows per attention block (both groups, post-gather)
E = 1024            # embedding
NQ = 512            # tq chunk (one PSUM bank of fp32)
NTQ = SEG // NQ     # 4 tq chunks per block
NTK = SEG // 128    # 16 tk chunks per block
ECH = E // 128      # 8 embedding chunks

_CACHE = {}


def _build_program():
    import concourse.bacc as bacc
    import concourse.mybir as mybir
    import concourse.tile as tile

    dt = mybir.dt
    nc = bacc.Bacc("TRN2", target_bir_lowering=False, debug=False,
                   enable_asserts=False)

    # ---- DRAM I/O (uniform across cores; host slices per core) ----
    # x tensors already in SBUF layout [128, 2(pos half) * 8(ec) * 1024(pos)]
    xs = {}
    for sel in ("a", "b"):
        for inp in ("q", "k", "v"):
            xs[(sel, inp)] = nc.dram_tensor(
                f"x{sel}_{inp}", [128, 2 * ECH * 1024], dt.bfloat16,
                kind="ExternalInput").ap()
    ws = {inp: nc.dram_tensor(f"w{inp}", [128, 384 * ECH], dt.bfloat16,
                              kind="ExternalInput").ap()
          for inp in ("q", "k", "v")}
    wo = nc.dram_tensor("wo", [128, 3 * E], dt.bfloat16,
                        kind="ExternalInput").ap()
    ball = nc.dram_tensor("ball", [128, 9], dt.float32,
                          kind="ExternalInput").ap()
    y0 = nc.dram_tensor("y0", [SEG, E], dt.bfloat16, kind="ExternalOutput").ap()
    y1 = nc.dram_tensor("y1", [SEG, E], dt.bfloat16, kind="ExternalOutput").ap()

    with tile.TileContext(nc) as tc:
        from contextlib import ExitStack
        with ExitStack() as ctx:
            const = ctx.enter_context(tc.tile_pool(name="const", bufs=1))
            wpool = ctx.enter_context(tc.tile_pool(name="wpool", bufs=1))
            xpool = ctx.enter_context(tc.tile_pool(name="xpool", bufs=3))
            vtp = ctx.enter_context(tc.tile_pool(name="vtp", bufs=2))
            qkt = ctx.enter_context(tc.tile_pool(name="qkt", bufs=1))
            vnat = ctx.enter_context(tc.tile_pool(name="vnat", bufs=1))
            otp = ctx.enter_context(tc.tile_pool(name="otp", bufs=1))
            ptp = ctx.enter_context(tc.tile_pool(name="ptp", bufs=2))
            dnp = ctx.enter_context(tc.tile_pool(name="dnp", bufs=1))
            dsp_p = ctx.enter_context(tc.tile_pool(name="dsp_p", bufs=1))
            rcq = ctx.enter_context(tc.tile_pool(name="rcq", bufs=2))
            rbp = ctx.enter_context(tc.tile_pool(name="rbp", bufs=2))
            rbtp = ctx.enter_context(tc.tile_pool(name="rbtp", bufs=1))
            ysb = ctx.enter_context(tc.tile_pool(name="ysb", bufs=2))
            ps_s = ctx.enter_context(
                tc.tile_pool(name="ps_s", bufs=2, space="PSUM"))
            ps_b = ctx.enter_context(
                tc.tile_pool(name="ps_b", bufs=1, space="PSUM"))
            ps_o = ctx.enter_context(
                tc.tile_pool(name="ps_o", bufs=2, space="PSUM"))

            # ---- constants: identity (PE transpose), causal chunk masks ----
            ident = const.tile([128, 128], dt.bfloat16, tag="ident")
            nc.gpsimd.memset(ident, 1.0)
            nc.gpsimd.affine_select(
                out=ident, in_=ident, compare_op=mybir.AluOpType.is_equal,
                fill=0.0, base=0, pattern=[[-1, 128]], channel_multiplier=1)
            # mask[di] ([128,1024], both 512-halves identical): for k-chunk
            # i = 4j + di vs q-chunk j: keep where f - p - 128*di >= 0
            masks = []
            for di in range(4):
                m = const.tile([128, 2 * NQ], dt.bfloat16, tag=f"mask{di}")
                nc.gpsimd.memset(m, 1.0)
                nc.gpsimd.affine_select(
                    out=m.rearrange("p (hh f) -> p hh f", hh=2),
                    in_=m.rearrange("p (hh f) -> p hh f", hh=2),
                    compare_op=mybir.AluOpType.is_ge,
                    fill=0.0, base=-128 * di, pattern=[[0, 2], [1, NQ]],
                    channel_multiplier=-1)
                masks.append(m)

            # ---- weights: one DMA per tensor (host pre-layouts) ----
            wTa = {}
            for inp in ("q", "k", "v"):
                t = wpool.tile([128, 384 * ECH], dt.bfloat16,
                               tag=f"wT_{inp}", name=f"wT_{inp}")
                nc.sync.dma_start(out=t, in_=ws[inp])
                wTa[inp] = t
            wT = {inp: [wTa[inp].rearrange("p (ec x) -> p ec x", x=384)
                        [:, :, 128 * p:128 * (p + 1)]
                        for p in range(3)] for inp in ("q", "k", "v")}
            woTa = wpool.tile([128, 3 * E], dt.bfloat16, tag="woT")
            nc.sync.dma_start(out=woTa, in_=wo)
            woT = [woTa[:, E * p:E * (p + 1)] for p in range(3)]
            ballt = wpool.tile([128, 9], dt.float32, tag="ball")
            nc.sync.dma_start(out=ballt, in_=ball)
            bsb = {}
            for i, inp in enumerate(("q", "k", "v")):
                for p in range(3):
                    bsb[(inp, p)] = ballt[:, 3 * i + p:3 * i + p + 1]

            # ---- persistent per-pair activations ----
            qT = [qkt.tile([128, SEG], dt.bfloat16, tag=f"qT{p}", name=f"qT{p}")
                  for p in range(3)]
            kT = [qkt.tile([128, SEG], dt.bfloat16, tag=f"kT{p}", name=f"kT{p}")
                  for p in range(3)]
            vn = [vnat.tile([128, NTK * 130], dt.bfloat16, tag=f"vn{p}",
                            name=f"vn{p}")
                  for p in range(3)]
            for p in range(3):
                ones_view = vn[p].rearrange("p (k x) -> p k x", x=130)
                nc.gpsimd.memset(ones_view[:, :, 64:65], 1.0)
                nc.gpsimd.memset(ones_view[:, :, 129:130], 1.0)
            oT = [otp.tile([128, SEG], dt.bfloat16, tag=f"oT{p}", name=f"oT{p}")
                  for p in range(3)]

            # ---- input loads: 2 DMAs (position halves) per tensor ----
            xt_tiles = {}
            xt_raw = {}
            HC = ECH * 1024     # columns per position half
            for sel, inp in (("a", "q"), ("a", "k"), ("a", "v"),
                             ("b", "q"), ("b", "k"), ("b", "v")):
                t = xpool.tile([128, 2 * HC], dt.bfloat16, tag="xt")
                xt_raw[(sel, inp)] = t
                xt_tiles[(sel, inp)] = t.rearrange(
                    "p (t2 ec s) -> p t2 ec s", t2=2, ec=ECH)

            def load_x_half(sel, inp, t2):
                t = xt_raw[(sel, inp)]
                nc.sync.dma_start(
                    out=t[:, HC * t2:HC * (t2 + 1)],
                    in_=xs[(sel, inp)][:, HC * t2:HC * (t2 + 1)])
            # xa halves interleaved (t2=0 of q,k,v first) so attention can
            # start after the first position-half is projected; xb after
            for t2 in range(2):
                for inp in ("q", "k", "v"):
                    load_x_half("a", inp, t2)
            for inp in ("q", "k", "v"):
                for t2 in range(2):
                    load_x_half("b", inp, t2)

            # ---- projection building blocks ----
            def proj_half(acc, inp, p, t2, half, sel):
                xt = xt_tiles[(sel, inp)]
                for ec in range(ECH):
                    nc.tensor.matmul(
                        acc[:, NQ * half:NQ * (half + 1)],
                        wT[inp][p][:, ec, :],
                        xt[:, t2, ec, NQ * half:NQ * (half + 1)],
                        start=(ec == 0), stop=(ec == ECH - 1))

            def vtrans_batch(p, i0, vtile):
                # transpose 4 [128,128] chunks of vT into V-natural slices
                ptr = ps_b.tile([128, 4 * 128], dt.bfloat16, tag="b",
                                name="ptr")
                for qq in range(4):
                    nc.tensor.transpose(
                        ptr[:, 128 * qq:128 * (qq + 1)],
                        vtile[:, 128 * (i0 + qq):128 * (i0 + qq + 1)], ident)
                src = ptr.rearrange("p (c h d) -> p c h d", c=4, h=2)
                dst = vn[p][:, 130 * i0:130 * (i0 + 4)].rearrange(
                    "p (c h x) -> p c h x", c=4, x=65)[:, :, :, 0:64]
                nc.vector.tensor_copy(dst, src)

            # ---- proj group: one [128,1024] output col-block ----
            def proj_group(inp, p, t2, dst, sel, act_bias):
                acc = (ps_s if act_bias else ps_b).tile(
                    [128, 2 * NQ], dt.float32, tag=("s" if act_bias else "b"),
                    name="proj")
                proj_half(acc, inp, p, t2, 0, sel)
                proj_half(acc, inp, p, t2, 1, sel)
                if act_bias:
                    nc.scalar.activation(
                        dst[:, 2 * NQ * t2:2 * NQ * (t2 + 1)], acc,
                        mybir.ActivationFunctionType.Identity,
                        bias=bsb[(inp, p)], scale=1.0)
                else:
                    nc.vector.tensor_scalar_add(
                        dst[:, 2 * NQ * t2:2 * NQ * (t2 + 1)], acc,
                        bsb[(inp, p)])

            # ---- Phase A: t2=0 projections for pairs 0,1 (pre-attention) --
            vts = {}
            for inp in ("q", "k", "v"):
                for p in (0, 1):
                    if inp == "v":
                        dst = vtp.tile([128, SEG], dt.bfloat16, tag="vT",
                                       name="vT")
                        vts[p] = dst
                    else:
                        dst = (qT if inp == "q" else kT)[p]
                    proj_group(inp, p, 0, dst, "a", True)
                    if inp == "v":
                        vtrans_batch(p, 0, dst)
                        vtrans_batch(p, 4, dst)

            # ---- filler machinery (global slot counter, absolute gates) --
            fillers = deque()   # (min_slot, thunk)
            slot_counter = [0]

            def slot_cb():
                slot_counter[0] += 1
                if fillers and fillers[0][0] <= slot_counter[0]:
                    fillers.popleft()[1]()

            def drain_fillers():
                while fillers:
                    fillers.popleft()[1]()

            # t2=1 projections for pairs 0,1 -> early attn(0) fillers
            for gi, (inp, p) in enumerate(
                    (("q", 0), ("q", 1), ("k", 0), ("k", 1),
                     ("v", 0), ("v", 1))):
                dst = vts[p] if inp == "v" else (qT if inp == "q" else kT)[p]
                fillers.append(
                    (2 + gi, lambda inp=inp, p=p, dst=dst:
                     proj_group(inp, p, 1, dst, "a", True)))
            for gi, (p, i0) in enumerate(((0, 8), (0, 12), (1, 8), (1, 12))):
                fillers.append(
                    (9 + gi, lambda p=p, i0=i0: vtrans_batch(p, i0, vts[p])))

            # pair-2 projections (xb) as later attn(0) fillers
            def make_projB_fillers():
                vtile = vtp.tile([128, SEG], dt.bfloat16, tag="vT",
                                 name="vtB")
                gates = {"q": 18, "k": 26, "v": 34}
                for inp in ("q", "k", "v"):
                    dst = {"q": qT[2], "k": kT[2], "v": vtile}[inp]
                    g = gates[inp]
                    for t2 in range(2):
                        fillers.append(
                            (g + 2 * t2, lambda inp=inp, t2=t2, dst=dst:
                             proj_group(inp, 2, t2, dst, "b", False)))
                for bi, i0 in enumerate((0, 4, 8, 12)):
                    fillers.append(
                        (38 + bi, lambda i0=i0: vtrans_batch(2, i0, vtile)))

            # out-projection chunk fillers
            def y_unit(ydram, pairs, m):
                def th():
                    accy = ps_b.tile([128, 2 * NQ], dt.float32, tag="b",
                                     name="accy")
                    for jc in range(2):
                        for idx, p in enumerate(pairs):
                            nc.tensor.matmul(
                                accy[:, NQ * jc:NQ * (jc + 1)],
                                oT[p][:, 128 * m:128 * (m + 1)],
                                woT[p][:, NQ * jc:NQ * (jc + 1)],
                                start=(idx == 0), stop=(idx == len(pairs) - 1))
                    t = ysb.tile([128, 2 * NQ], dt.bfloat16, tag="ysb")
                    nc.vector.tensor_copy(t, accy)
                    nc.sync.dma_start(
                        out=ydram[128 * m:128 * (m + 1), :], in_=t)
                return th

            # ---- normalization (DMA-scatter batched reciprocal) ----
            def drow(j, h):
                return 32 * (2 * (j % 2) + h), NQ * (j // 2)

            dens = [dnp.tile([128, 2 * NQ], dt.float32, tag="dens",
                             name=f"dens{p}") for p in range(3)]
            dsp = [dsp_p.tile([128, 32], dt.float32, tag=f"dsp{p}",
                              name=f"dsp{p}") for p in range(3)]
            rsp = [dsp_p.tile([128, 32], dt.float32, tag=f"rsp{p}",
                              name=f"rsp{p}") for p in range(3)]

            def recip_batch(p, us):
                def th():
                    for u in us:
                        r, c = drow(u // 2, u % 2)
                        nc.sync.dma_start(
                            out=dsp[p][:, 4 * u:4 * (u + 1)],
                            in_=dens[p][r:r + 1, c:c + NQ])
                    u0, u1 = us[0], us[-1] + 1
                    nc.vector.reciprocal(rsp[p][:, 4 * u0:4 * u1],
                                         dsp[p][:, 4 * u0:4 * u1])
                return th

            def norm_bc_unit(p, j, rb2s):
                def th():
                    rb2 = rbp.tile([128, NQ], dt.float32, tag="rb")
                    rb2s[j] = rb2
                    for h in range(2):
                        u = 2 * j + h
                        rc = rcq.tile([1, NQ], dt.float32, tag="rc")
                        nc.sync.dma_start(out=rc,
                                          in_=rsp[p][:, 4 * u:4 * (u + 1)])
                        if h == 0:
                            nc.gpsimd.partition_broadcast(rb2[0:64, :], rc)
                        else:
                            rbt = rbtp.tile([64, NQ], dt.float32,
                                            tag="rbt")
                            nc.gpsimd.partition_broadcast(rbt, rc)
                            nc.sync.dma_start(out=rb2[64:128, :], in_=rbt)
                return th

            def norm_mul_unit(p, j, rb2s):
                def th():
                    sl = oT[p][:, NQ * j:NQ * (j + 1)]
                    nc.vector.tensor_mul(sl, sl, rb2s.pop(j))
                return th

            def norm_fillers(p, base, js):
                rb2s = {}
                ents = [(base, recip_batch(p, [u for j in js
                                               for u in (2 * j, 2 * j + 1)]))]
                for i, j in enumerate(js):
                    ents.append((base + 2 + i, norm_bc_unit(p, j, rb2s)))
                    ents.append((base + 4 + i, norm_mul_unit(p, j, rb2s)))
                ents.sort(key=lambda e: e[0])
                return ents

            # ---- attention: flat chunk stream with cross-boundary S
            # lookahead (keeps ACT fed across j/pair boundaries) ----
            def s_emit(p, j, ci):
                s = ps_s.tile([128, 2 * NQ], dt.float32, tag="s", name="s")
                nc.tensor.matmul(
                    s[:, 0:NQ],
                    kT[p][0:64, 128 * ci:128 * (ci + 1)],
                    qT[p][0:64, NQ * j:NQ * (j + 1)],
                    start=True, stop=True)
                nc.tensor.matmul(
                    s[:, NQ:2 * NQ],
                    kT[p][64:128, 128 * ci:128 * (ci + 1)],
                    qT[p][64:128, NQ * j:NQ * (j + 1)],
                    start=True, stop=True)
                return s

            chunks = []         # flat stream: (p, j, ci, nchunks)
            hooks = {}          # (p, j) -> after_j hook
            for p in range(3):
                for j in range(NTQ):
                    n = 4 * (j + 1)
                    for ci in range(n):
                        chunks.append((p, j, ci, n))

            def attn_all():
                accs = {}
                s_tiles = {}
                first = chunks[0]
                s_tiles[first[:3]] = s_emit(*first[:3])
                for idx, (p, j, ci, n) in enumerate(chunks):
                    if ci == 0:
                        acc0 = ps_o.tile([128, NQ], dt.float32, tag="acc",
                                         name="acc0")
                        acc1 = ps_o.tile([128, NQ], dt.float32, tag="acc",
                                         name="acc1")
                        accs[(p, j)] = (acc0, acc1)
                    acc0, acc1 = accs[(p, j)]
                    s = s_tiles.pop((p, j, ci))
                    pt = ptp.tile([128, 2 * NQ], dt.bfloat16, tag="pt",
                                  name="pt")
                    nc.scalar.activation(
                        pt, s, mybir.ActivationFunctionType.Exp,
                        bias=0.0, scale=0.125)
                    di = ci - 4 * j
                    if di >= 0:
                        nc.vector.tensor_mul(pt, pt, masks[di])
                    if idx + 1 < len(chunks):
                        nxt = chunks[idx + 1]
                        s_tiles[nxt[:3]] = s_emit(*nxt[:3])
                    last = (ci == n - 1)
                    nc.tensor.matmul(
                        acc0[0:65, :],
                        vn[p][:, 130 * ci:130 * ci + 65],
                        pt[:, 0:NQ], start=(ci == 0), stop=last)
                    nc.tensor.matmul(
                        acc1[0:65, :],
                        vn[p][:, 130 * ci + 65:130 * ci + 130],
                        pt[:, NQ:2 * NQ], start=(ci == 0), stop=last)
                    slot_cb()
                    if last:
                        for h, acc in enumerate((acc0, acc1)):
                            nc.vector.tensor_copy(
                                oT[p][64 * h:64 * h + 64,
                                      NQ * j:NQ * (j + 1)],
                                acc[0:64, :])
                            r, c = drow(j, h)
                            nc.vector.tensor_copy(
                                dens[p][r:r + 1, c:c + NQ], acc[64:65, :])
                        accs.pop((p, j))
                        hk = hooks.get((p, j))
                        if hk is not None:
                            hk()

            # norm/y filler schedule (absolute slots: pair p spans
            # [40p, 40p+40))
            def queue_pair0_norm():
                fillers.extend(norm_fillers(0, 40, [0, 1, 2, 3]))
            hooks[(0, 3)] = queue_pair0_norm

            def queue_pair1_norm():
                fillers.extend(norm_fillers(1, 80, [0, 1, 2, 3]))
                for m in range(16):
                    fillers.append((89 + m, y_unit(y0, (0, 1), m)))
            hooks[(1, 3)] = queue_pair1_norm

            def queue_pair2_mid():
                fillers.extend(norm_fillers(2, 105, [0, 1, 2]))
                for m in range(12):
                    fillers.append((113 + m, y_unit(y1, (2,), m)))
            hooks[(2, 2)] = queue_pair2_mid

            make_projB_fillers()
            attn_all()
            drain_fillers()
            # tail: j3 norm + last y1 units
            rb2s_t = {}
            recip_batch(2, [6, 7])()
            norm_bc_unit(2, 3, rb2s_t)()
            norm_mul_unit(2, 3, rb2s_t)()
            for m in range(12, 16):
                y_unit(y1, (2,), m)()

    nc.compile()
    return nc


def _get_program():
    if "nc" not in _CACHE:
        _CACHE["nc"] = _build_program()
    return _CACHE["nc"]


def _sbuf_layout(xT):
    """[1024, 2048] (e, pos) -> [128, 2*8*1024]: pos-half major, ec, pos."""
    # [ec, 128, t2, 1024] -> [128, t2, ec, 1024]
    return np.ascontiguousarray(
        xT.reshape(ECH, 128, 2, 1024).transpose(1, 2, 0, 3).reshape(
            128, 2 * ECH * 1024))


def _prep_inputs(query, key, value, Wq, bq, Wk, bk, Wv, bv, Wo, bo):
    """Build the 8 per-core input maps (host-side slicing + bf16 cast)."""
    q = np.asarray(query, np.float32).reshape(8192, 1024).astype(BF16)
    k = np.asarray(key, np.float32).reshape(8192, 1024).astype(BF16)
    v = np.asarray(value, np.float32).reshape(8192, 1024).astype(BF16)
    wq = np.asarray(Wq, np.float32).astype(BF16)
    wk = np.asarray(Wk, np.float32).astype(BF16)
    wv = np.asarray(Wv, np.float32).astype(BF16)
    wo_f = np.asarray(Wo, np.float32).astype(BF16)
    bqf = np.asarray(bq, np.float32)
    bkf = np.asarray(bk, np.float32)
    bvf = np.asarray(bv, np.float32)

    qT, kT, vT = q.T, k.T, v.T  # [1024, 8192] views
    in_maps = []
    for c in range(8):
        a, sc, b, rc = c % 2, c // 2, c % 4, c // 4
        rows_g0 = slice(2048 * sc, 2048 * (sc + 1))
        rows_g1 = slice(4096 * rc + 1, 4096 * (rc + 1), 2)
        hrows = np.r_[256 * a:256 * a + 256, 512 + 128 * b:512 + 128 * b + 128]

        def wlay(w):
            # [1024, 384] -> [128, 8*384] (ec-blocked)
            return np.ascontiguousarray(
                w.reshape(ECH, 128, 384).transpose(1, 0, 2).reshape(128, -1))

        wov = np.ascontiguousarray(wo_f[:, hrows].T)  # [384, 1024]
        ballv = np.stack([np.ascontiguousarray(bf[hrows]).reshape(3, 128)
                          for bf in (bqf, bkf, bvf)], axis=0)  # [3,3,128]
        # ball[128, 9]: col 3i+p = input i, pair p
        ballv = np.ascontiguousarray(ballv.reshape(9, 128).T)
        m = {
            "xa_q": _sbuf_layout(np.ascontiguousarray(qT[:, rows_g0])),
            "xa_k": _sbuf_layout(np.ascontiguousarray(kT[:, rows_g0])),
            "xa_v": _sbuf_layout(np.ascontiguousarray(vT[:, rows_g0])),
            "xb_q": _sbuf_layout(np.ascontiguousarray(qT[:, rows_g1])),
            "xb_k": _sbuf_layout(np.ascontiguousarray(kT[:, rows_g1])),
            "xb_v": _sbuf_layout(np.ascontiguousarray(vT[:, rows_g1])),
            "wq": wlay(np.ascontiguousarray(wq[hrows].T)),
            "wk": wlay(np.ascontiguousarray(wk[hrows].T)),
            "wv": wlay(np.ascontiguousarray(wv[hrows].T)),
            "wo": np.ascontiguousarray(
                wov.reshape(3, 128, 1024).transpose(1, 0, 2).reshape(128, -1)),
            "ball": ballv,
        }
        in_maps.append(m)
    return in_maps


def _combine(results, bo):
    y = np.zeros((8192, 1024), np.float32)
    for c in range(8):
        sc, rc = c // 2, c // 4
        y[2048 * sc:2048 * (sc + 1)] += results[c]["y0"].astype(np.float32)
        y[4096 * rc + 1:4096 * (rc + 1):2] += results[c]["y1"].astype(np.float32)
    y += np.asarray(bo, np.float32)
    return y.reshape(1, 8192, 1024)


def kernel(query, key, value, Wq, bq, Wk, bk, Wv, bv, Wo, bo,
           _trace=False, _trace_cores=None):
    from concourse import bass_utils
    nc = _get_program()
    in_maps = _prep_inputs(query, key, value, Wq, bq, Wk, bk, Wv, bv, Wo, bo)
    res = bass_utils.run_bass_kernel_spmd(
        nc, in_maps, core_ids=list(range(8)),
        trace=_trace, trace_cores=_trace_cores)
    _CACHE["last_results"] = res
    return _combine(res.results, bo)


# revision 20
# speedup vs baseline: 1.2263x; 1.0007x over previous
"""BlockSparseRingMultiheadDilatedAttention Trainium2 kernel (v3).

Problem (hardcoded): B=1, N=8192, E=1024, H=16 heads, D=64.
Two dilated groups: g0 = heads 0-7, seg 2048, dilation 1;
                    g1 = heads 8-15, seg 4096, dilation 2, offset 1 (odd positions).
Causal within each (gathered) segment.

Sharding over 8 cores (uniform SPMD program, per-core data):
  core c: a = c%2, sc = c//2, b = c%4, rc = c//4
    g0: seg sc (rows 2048*sc .. +2048), heads 4a..4a+4   (4 blocks of [2048 x 2048])
    g1: seg rc odd rows (gathered, 2048 rows), heads 8+2b..+2 (2 blocks)
  Host pre-slices inputs (bf16 cast, odd-row gather, weight head slices,
  SBUF-layout rearrange) so the device program is identical on every core.
  Host sums the per-core partial output projections (disjoint head
  contributions, bf16) and adds bo.

v3 device dataflow:
  - few LARGE input DMAs (2 per x tensor, 1 per weight tensor) in final SBUF
    layout -- per-DMA fixed costs gated startup in v1/v2 (single HW queue,
    ~2us/DMA completion latency)
  - attention processes BOTH heads of a pair per chunk: two row-tiled
    (tile_position (0,0)/(64,0)) K=64 S-matmuls run concurrently in the PE
    array, land in one 2-bank PSUM tile -> ONE exp per [128,1024]
  - softmax: denominators from the appended-ones row of the PV matmul;
    numerators copied UNNORMALIZED to oT (releases PSUM fast); denominator
    rows gathered onto 8 partitions of one tile -> ONE batched DVE
    reciprocal per pair; broadcast+normalize muls run as fillers inside the
    NEXT pair's window (keeps the DVE FIFO from stalling PE -> HAM warm)
  - pair-2 qkv projections + V transposes are fillers inside pair 0/1's
    ACT(exp)-limited windows; y0/y1 out-proj chunks stream inside pair 2
  - outputs in bf16 (host combines in fp32)
"""

import numpy as np
import ml_dtypes
from collections import deque

BF16 = ml_dtypes.bfloat16

SEG = 2048          # rows per attention block (both groups, post-gather)
E = 1024            # embedding
NQ = 512            # tq chunk (one PSUM bank of fp32)
NTQ = SEG // NQ     # 4 tq chunks per block
NTK = SEG // 128    # 16 tk chunks per block
ECH = E // 128      # 8 embedding chunks

_CACHE = {}


def _build_program():
    import concourse.bacc as bacc
    import concourse.mybir as mybir
    import concourse.tile as tile

    dt = mybir.dt
    nc = bacc.Bacc("TRN2", target_bir_lowering=False, debug=False,
                   enable_asserts=False)

    # ---- DRAM I/O (uniform across cores; host slices per core) ----
    # x tensors already in SBUF layout [128, 2(pos half) * 8(ec) * 1024(pos)]
    xs = {}
    for sel in ("a", "b"):
        for inp in ("q", "k", "v"):
            xs[(sel, inp)] = nc.dram_tensor(
                f"x{sel}_{inp}", [128, 2 * ECH * 1024], dt.bfloat16,
                kind="ExternalInput").ap()
    ws = {inp: nc.dram_tensor(f"w{inp}", [128, 384 * ECH], dt.bfloat16,
                              kind="ExternalInput").ap()
          for inp in ("q", "k", "v")}
    wo = nc.dram_tensor("wo", [128, 3 * E], dt.bfloat16,
                        kind="ExternalInput").ap()
    ball = nc.dram_tensor("ball", [128, 9], dt.float32,
                          kind="ExternalInput").ap()
    y0 = nc.dram_tensor("y0", [SEG, E], dt.bfloat16, kind="ExternalOutput").ap()
    y1 = nc.dram_tensor("y1", [SEG, E], dt.bfloat16, kind="ExternalOutput").ap()

    with tile.TileContext(nc) as tc:
        from contextlib import ExitStack
        with ExitStack() as ctx:
            const = ctx.enter_context(tc.tile_pool(name="const", bufs=1))
            wpool = ctx.enter_context(tc.tile_pool(name="wpool", bufs=1))
            xpool = ctx.enter_context(tc.tile_pool(name="xpool", bufs=3))
            vtp = ctx.enter_context(tc.tile_pool(name="vtp", bufs=2))
            qkt = ctx.enter_context(tc.tile_pool(name="qkt", bufs=1))
            vnat = ctx.enter_context(tc.tile_pool(name="vnat", bufs=1))
            otp = ctx.enter_context(tc.tile_pool(name="otp", bufs=1))
            ptp = ctx.enter_context(tc.tile_pool(name="ptp", bufs=2))
            dnp = ctx.enter_context(tc.tile_pool(name="dnp", bufs=1))
            dsp_p = ctx.enter_context(tc.tile_pool(name="dsp_p", bufs=2))
            rcq = ctx.enter_context(tc.tile_pool(name="rcq", bufs=2))
            rbp = ctx.enter_context(tc.tile_pool(name="rbp", bufs=2))
            rbtp = ctx.enter_context(tc.tile_pool(name="rbtp", bufs=1))
            ysb = ctx.enter_context(tc.tile_pool(name="ysb", bufs=2))
            ps_s = ctx.enter_context(
                tc.tile_pool(name="ps_s", bufs=2, space="PSUM"))
            ps_b = ctx.enter_context(
                tc.tile_pool(name="ps_b", bufs=1, space="PSUM"))
            ps_o = ctx.enter_context(
                tc.tile_pool(name="ps_o", bufs=2, space="PSUM"))

            # ---- constants: identity (PE transpose), causal chunk masks ----
            ident = const.tile([128, 128], dt.bfloat16, tag="ident")
            nc.gpsimd.memset(ident, 1.0)
            nc.gpsimd.affine_select(
                out=ident, in_=ident, compare_op=mybir.AluOpType.is_equal,
                fill=0.0, base=0, pattern=[[-1, 128]], channel_multiplier=1)
            # mask[di] ([128,1024], both 512-halves identical): for k-chunk
            # i = 4j + di vs q-chunk j: keep where f - p - 128*di >= 0
            masks = []
            for di in range(4):
                m = const.tile([128, 2 * NQ], dt.bfloat16, tag=f"mask{di}")
                nc.gpsimd.memset(m, 1.0)
                nc.gpsimd.affine_select(
                    out=m.rearrange("p (hh f) -> p hh f", hh=2),
                    in_=m.rearrange("p (hh f) -> p hh f", hh=2),
                    compare_op=mybir.AluOpType.is_ge,
                    fill=0.0, base=-128 * di, pattern=[[0, 2], [1, NQ]],
                    channel_multiplier=-1)
                masks.append(m)

            # ---- weights: one DMA per tensor (host pre-layouts) ----
            wTa = {}
            for inp in ("q", "k", "v"):
                t = wpool.tile([128, 384 * ECH], dt.bfloat16,
                               tag=f"wT_{inp}", name=f"wT_{inp}")
                nc.sync.dma_start(out=t, in_=ws[inp])
                wTa[inp] = t
            wT = {inp: [wTa[inp].rearrange("p (ec x) -> p ec x", x=384)
                        [:, :, 128 * p:128 * (p + 1)]
                        for p in range(3)] for inp in ("q", "k", "v")}
            woTa = wpool.tile([128, 3 * E], dt.bfloat16, tag="woT")
            nc.sync.dma_start(out=woTa, in_=wo)
            woT = [woTa[:, E * p:E * (p + 1)] for p in range(3)]
            ballt = wpool.tile([128, 9], dt.float32, tag="ball")
            nc.sync.dma_start(out=ballt, in_=ball)
            bsb = {}
            for i, inp in enumerate(("q", "k", "v")):
                for p in range(3):
                    bsb[(inp, p)] = ballt[:, 3 * i + p:3 * i + p + 1]

            # ---- persistent per-pair activations ----
            qT = [qkt.tile([128, SEG], dt.bfloat16, tag=f"qT{p}", name=f"qT{p}")
                  for p in range(3)]
            kT = [qkt.tile([128, SEG], dt.bfloat16, tag=f"kT{p}", name=f"kT{p}")
                  for p in range(3)]
            vn = [vnat.tile([128, NTK * 130], dt.bfloat16, tag=f"vn{p}",
                            name=f"vn{p}")
                  for p in range(3)]
            for p in range(3):
                ones_view = vn[p].rearrange("p (k x) -> p k x", x=130)
                nc.gpsimd.memset(ones_view[:, :, 64:65], 1.0)
                nc.gpsimd.memset(ones_view[:, :, 129:130], 1.0)
            oT = [otp.tile([128, SEG], dt.bfloat16, tag=f"oT{p}", name=f"oT{p}")
                  for p in range(3)]

            # ---- input loads: 2 DMAs (position halves) per tensor ----
            xt_tiles = {}
            xt_raw = {}
            HC = ECH * 1024     # columns per position half
            for sel, inp in (("a", "q"), ("a", "k"), ("a", "v"),
                             ("b", "q"), ("b", "k"), ("b", "v")):
                t = xpool.tile([128, 2 * HC], dt.bfloat16, tag="xt")
                xt_raw[(sel, inp)] = t
                xt_tiles[(sel, inp)] = t.rearrange(
                    "p (t2 ec s) -> p t2 ec s", t2=2, ec=ECH)

            def load_x_half(sel, inp, t2):
                t = xt_raw[(sel, inp)]
                nc.sync.dma_start(
                    out=t[:, HC * t2:HC * (t2 + 1)],
                    in_=xs[(sel, inp)][:, HC * t2:HC * (t2 + 1)])
            # xa halves interleaved (t2=0 of q,k,v first) so attention can
            # start after the first position-half is projected; xb after
            for t2 in range(2):
                for inp in ("q", "k", "v"):
                    load_x_half("a", inp, t2)
            for inp in ("q", "k", "v"):
                for t2 in range(2):
                    load_x_half("b", inp, t2)

            # ---- projection building blocks ----
            def proj_half(acc, inp, p, t2, half, sel):
                xt = xt_tiles[(sel, inp)]
                for ec in range(ECH):
                    nc.tensor.matmul(
                        acc[:, NQ * half:NQ * (half + 1)],
                        wT[inp][p][:, ec, :],
                        xt[:, t2, ec, NQ * half:NQ * (half + 1)],
                        start=(ec == 0), stop=(ec == ECH - 1))

            def vtrans_batch(p, i0, vtile):
                # transpose 4 [128,128] chunks of vT into V-natural slices
                ptr = ps_b.tile([128, 4 * 128], dt.bfloat16, tag="b",
                                name="ptr")
                for qq in range(4):
                    nc.tensor.transpose(
                        ptr[:, 128 * qq:128 * (qq + 1)],
                        vtile[:, 128 * (i0 + qq):128 * (i0 + qq + 1)], ident)
                src = ptr.rearrange("p (c h d) -> p c h d", c=4, h=2)
                dst = vn[p][:, 130 * i0:130 * (i0 + 4)].rearrange(
                    "p (c h x) -> p c h x", c=4, x=65)[:, :, :, 0:64]
                nc.vector.tensor_copy(dst, src)

            # ---- proj group: one [128,1024] output col-block ----
            def proj_group(inp, p, t2, dst, sel, act_bias):
                acc = (ps_s if act_bias else ps_b).tile(
                    [128, 2 * NQ], dt.float32, tag=("s" if act_bias else "b"),
                    name="proj")
                proj_half(acc, inp, p, t2, 0, sel)
                proj_half(acc, inp, p, t2, 1, sel)
                if act_bias:
                    nc.scalar.activation(
                        dst[:, 2 * NQ * t2:2 * NQ * (t2 + 1)], acc,
                        mybir.ActivationFunctionType.Identity,
                        bias=bsb[(inp, p)], scale=1.0)
                else:
                    nc.vector.tensor_scalar_add(
                        dst[:, 2 * NQ * t2:2 * NQ * (t2 + 1)], acc,
                        bsb[(inp, p)])

            # ---- Phase A: t2=0 projections for pairs 0,1 (pre-attention) --
            vts = {}
            for inp in ("q", "k", "v"):
                for p in (0, 1):
                    if inp == "v":
                        dst = vtp.tile([128, SEG], dt.bfloat16, tag="vT",
                                       name="vT")
                        vts[p] = dst
                    else:
                        dst = (qT if inp == "q" else kT)[p]
                    proj_group(inp, p, 0, dst, "a", True)
                    if inp == "v":
                        vtrans_batch(p, 0, dst)
                        vtrans_batch(p, 4, dst)

            # ---- filler machinery (global slot counter, absolute gates) --
            fillers = deque()   # (min_slot, thunk)
            slot_counter = [0]

            def slot_cb():
                slot_counter[0] += 1
                if fillers and fillers[0][0] <= slot_counter[0]:
                    fillers.popleft()[1]()

            def drain_fillers():
                while fillers:
                    fillers.popleft()[1]()

            # t2=1 projections for pairs 0,1 -> early attn(0) fillers
            for gi, (inp, p) in enumerate(
                    (("q", 0), ("q", 1), ("k", 0), ("k", 1),
                     ("v", 0), ("v", 1))):
                dst = vts[p] if inp == "v" else (qT if inp == "q" else kT)[p]
                fillers.append(
                    (2 + gi, lambda inp=inp, p=p, dst=dst:
                     proj_group(inp, p, 1, dst, "a", True)))
            for gi, (p, i0) in enumerate(((0, 8), (0, 12), (1, 8), (1, 12))):
                fillers.append(
                    (9 + gi, lambda p=p, i0=i0: vtrans_batch(p, i0, vts[p])))

            # pair-2 projections (xb) as later attn(0) fillers
            def make_projB_fillers():
                vtile = vtp.tile([128, SEG], dt.bfloat16, tag="vT",
                                 name="vtB")
                gates = {"q": 18, "k": 26, "v": 34}
                for inp in ("q", "k", "v"):
                    dst = {"q": qT[2], "k": kT[2], "v": vtile}[inp]
                    g = gates[inp]
                    for t2 in range(2):
                        fillers.append(
                            (g + 2 * t2, lambda inp=inp, t2=t2, dst=dst:
                             proj_group(inp, 2, t2, dst, "b", False)))
                for bi, i0 in enumerate((0, 4, 8, 12)):
                    fillers.append(
                        (38 + bi, lambda i0=i0: vtrans_batch(2, i0, vtile)))

            # out-projection chunk fillers
            def y_unit(ydram, pairs, m, cast_on_act=False):
                def th():
                    accy = ps_b.tile([128, 2 * NQ], dt.float32, tag="b",
                                     name="accy")
                    for jc in range(2):
                        for idx, p in enumerate(pairs):
                            nc.tensor.matmul(
                                accy[:, NQ * jc:NQ * (jc + 1)],
                                oT[p][:, 128 * m:128 * (m + 1)],
                                woT[p][:, NQ * jc:NQ * (jc + 1)],
                                start=(idx == 0), stop=(idx == len(pairs) - 1))
                    t = ysb.tile([128, 2 * NQ], dt.bfloat16, tag="ysb")
                    if cast_on_act:
                        nc.scalar.copy(t, accy)
                    else:
                        nc.vector.tensor_copy(t, accy)
                    nc.sync.dma_start(
                        out=ydram[128 * m:128 * (m + 1), :], in_=t)
                return th

            # ---- normalization (DMA-scatter batched reciprocal) ----
            def drow(j, h):
                return 32 * (2 * (j % 2) + h), NQ * (j // 2)

            dens = [dnp.tile([128, 2 * NQ], dt.bfloat16, tag="dens",
                             name=f"dens{p}") for p in range(3)]
            dsp = [dsp_p.tile([32, 8 * 16], dt.bfloat16, tag="dsp",
                              name=f"dsp{p}") for p in range(3)]
            rsp = [dsp_p.tile([32, 8 * 16], dt.float32, tag="rsp",
                              name=f"rsp{p}") for p in range(3)]

            def recip_batch(p, us):
                # spread each den row over 32 partitions (64B DMA lines) so
                # ONE short-free-dim reciprocal covers the batch
                def th():
                    for u in us:
                        r, c = drow(u // 2, u % 2)
                        nc.sync.dma_start(
                            out=dsp[p][:, 16 * u:16 * (u + 1)],
                            in_=dens[p][r:r + 1, c:c + NQ])
                    u0, u1 = us[0], us[-1] + 1
                    nc.vector.reciprocal(rsp[p][:, 16 * u0:16 * u1],
                                         dsp[p][:, 16 * u0:16 * u1])
                return th

            def norm_bc_unit(p, j, rb2s):
                def th():
                    rb2 = rbp.tile([128, NQ], dt.float32, tag="rb")
                    rb2s[j] = rb2
                    for h in range(2):
                        u = 2 * j + h
                        rc = rcq.tile([1, NQ], dt.float32, tag="rc")
                        nc.sync.dma_start(out=rc,
                                          in_=rsp[p][:, 16 * u:16 * (u + 1)])
                        if h == 0:
                            nc.gpsimd.partition_broadcast(rb2[0:64, :], rc)
                        else:
                            rbt = rbtp.tile([64, NQ], dt.float32,
                                            tag="rbt")
                            nc.gpsimd.partition_broadcast(rbt, rc)
                            nc.sync.dma_start(out=rb2[64:128, :], in_=rbt)
                return th

            def norm_mul_unit(p, j, rb2s):
                def th():
                    sl = oT[p][:, NQ * j:NQ * (j + 1)]
                    nc.vector.tensor_mul(sl, sl, rb2s.pop(j))
                return th

            def norm_fillers(p, base, js):
                rb2s = {}
                ents = [(base, recip_batch(p, [u for j in js
                                               for u in (2 * j, 2 * j + 1)]))]
                for i, j in enumerate(js):
                    ents.append((base + 2 + i, norm_bc_unit(p, j, rb2s)))
                    ents.append((base + 4 + i, norm_mul_unit(p, j, rb2s)))
                ents.sort(key=lambda e: e[0])
                return ents

            # ---- attention: flat chunk stream with cross-boundary S
            # lookahead (keeps ACT fed across j/pair boundaries) ----
            def s_emit(p, j, ci):
                s = ps_s.tile([128, 2 * NQ], dt.float32, tag="s", name="s")
                nc.tensor.matmul(
                    s[:, 0:NQ],
                    kT[p][0:64, 128 * ci:128 * (ci + 1)],
                    qT[p][0:64, NQ * j:NQ * (j + 1)],
                    start=True, stop=True)
                nc.tensor.matmul(
                    s[:, NQ:2 * NQ],
                    kT[p][64:128, 128 * ci:128 * (ci + 1)],
                    qT[p][64:128, NQ * j:NQ * (j + 1)],
                    start=True, stop=True)
                return s

            chunks = []         # flat stream: (p, j, ci, nchunks)
            hooks = {}          # (p, j) -> after_j hook
            for p in range(3):
                for j in range(NTQ):
                    n = 4 * (j + 1)
                    for ci in range(n):
                        chunks.append((p, j, ci, n))

            def attn_all():
                accs = {}
                s_tiles = {}
                first = chunks[0]
                s_tiles[first[:3]] = s_emit(*first[:3])
                for idx, (p, j, ci, n) in enumerate(chunks):
                    if ci == 0:
                        acc0 = ps_o.tile([128, NQ], dt.float32, tag="acc",
                                         name="acc0")
                        acc1 = ps_o.tile([128, NQ], dt.float32, tag="acc",
                                         name="acc1")
                        accs[(p, j)] = (acc0, acc1)
                    acc0, acc1 = accs[(p, j)]
                    s = s_tiles.pop((p, j, ci))
                    pt = ptp.tile([128, 2 * NQ], dt.bfloat16, tag="pt",
                                  name="pt")
                    nc.scalar.activation(
                        pt, s, mybir.ActivationFunctionType.Exp,
                        bias=0.0, scale=0.125)
                    di = ci - 4 * j
                    if di >= 0:
                        nc.vector.tensor_mul(pt, pt, masks[di])
                    if idx + 1 < len(chunks):
                        nxt = chunks[idx + 1]
                        s_tiles[nxt[:3]] = s_emit(*nxt[:3])
                    last = (ci == n - 1)
                    nc.tensor.matmul(
                        acc0[0:65, :],
                        vn[p][:, 130 * ci:130 * ci + 65],
                        pt[:, 0:NQ], start=(ci == 0), stop=last)
                    nc.tensor.matmul(
                        acc1[0:65, :],
                        vn[p][:, 130 * ci + 65:130 * ci + 130],
                        pt[:, NQ:2 * NQ], start=(ci == 0), stop=last)
                    slot_cb()
                    if last:
                        for h, acc in enumerate((acc0, acc1)):
                            nc.vector.tensor_copy(
                                oT[p][64 * h:64 * h + 64,
                                      NQ * j:NQ * (j + 1)],
                                acc[0:64, :])
                            r, c = drow(j, h)
                            nc.vector.tensor_copy(
                                dens[p][r:r + 1, c:c + NQ], acc[64:65, :])
                        accs.pop((p, j))
                        hk = hooks.get((p, j))
                        if hk is not None:
                            hk()

            # norm/y filler schedule (absolute slots: pair p spans
            # [40p, 40p+40))
            def queue_pair0_norm():
                fillers.extend(norm_fillers(0, 40, [0, 1, 2, 3]))
            hooks[(0, 3)] = queue_pair0_norm

            def queue_pair1_norm():
                fillers.extend(norm_fillers(1, 80, [0, 1, 2, 3]))
                for m in range(16):
                    fillers.append((89 + m, y_unit(y0, (0, 1), m)))
            hooks[(1, 3)] = queue_pair1_norm

            def queue_pair2_mid():
                fillers.extend(norm_fillers(2, 105, [0, 1, 2]))
                for m in range(12):
                    fillers.append((113 + m, y_unit(y1, (2,), m)))
            hooks[(2, 2)] = queue_pair2_mid

            make_projB_fillers()
            attn_all()
            drain_fillers()
            # tail: j3 norm + last y1 units
            rb2s_t = {}
            recip_batch(2, [6, 7])()
            norm_bc_unit(2, 3, rb2s_t)()
            norm_mul_unit(2, 3, rb2s_t)()
            for m in range(12, 16):
                y_unit(y1, (2,), m, cast_on_act=True)()

    nc.compile()
    return nc


def _get_program():
    if "nc" not in _CACHE:
        _CACHE["nc"] = _build_program()
    return _CACHE["nc"]


def _sbuf_layout(xT):
    """[1024, 2048] (e, pos) -> [128, 2*8*1024]: pos-half major, ec, pos."""
    # [ec, 128, t2, 1024] -> [128, t2, ec, 1024]
    return np.ascontiguousarray(
        xT.reshape(ECH, 128, 2, 1024).transpose(1, 2, 0, 3).reshape(
            128, 2 * ECH * 1024))


def _prep_inputs(query, key, value, Wq, bq, Wk, bk, Wv, bv, Wo, bo):
    """Build the 8 per-core input maps (host-side slicing + bf16 cast)."""
    q = np.asarray(query, np.float32).reshape(8192, 1024).astype(BF16)
    k = np.asarray(key, np.float32).reshape(8192, 1024).astype(BF16)
    v = np.asarray(value, np.float32).reshape(8192, 1024).astype(BF16)
    wq = np.asarray(Wq, np.float32).astype(BF16)
    wk = np.asarray(Wk, np.float32).astype(BF16)
    wv = np.asarray(Wv, np.float32).astype(BF16)
    wo_f = np.asarray(Wo, np.float32).astype(BF16)
    bqf = np.asarray(bq, np.float32)
    bkf = np.asarray(bk, np.float32)
    bvf = np.asarray(bv, np.float32)

    qT, kT, vT = q.T, k.T, v.T  # [1024, 8192] views
    in_maps = []
    for c in range(8):
        a, sc, b, rc = c % 2, c // 2, c % 4, c // 4
        rows_g0 = slice(2048 * sc, 2048 * (sc + 1))
        rows_g1 = slice(4096 * rc + 1, 4096 * (rc + 1), 2)
        hrows = np.r_[256 * a:256 * a + 256, 512 + 128 * b:512 + 128 * b + 128]

        def wlay(w):
            # [1024, 384] -> [128, 8*384] (ec-blocked)
            return np.ascontiguousarray(
                w.reshape(ECH, 128, 384).transpose(1, 0, 2).reshape(128, -1))

        wov = np.ascontiguousarray(wo_f[:, hrows].T)  # [384, 1024]
        ballv = np.stack([np.ascontiguousarray(bf[hrows]).reshape(3, 128)
                          for bf in (bqf, bkf, bvf)], axis=0)  # [3,3,128]
        # ball[128, 9]: col 3i+p = input i, pair p
        ballv = np.ascontiguousarray(ballv.reshape(9, 128).T)
        m = {
            "xa_q": _sbuf_layout(np.ascontiguousarray(qT[:, rows_g0])),
            "xa_k": _sbuf_layout(np.ascontiguousarray(kT[:, rows_g0])),
            "xa_v": _sbuf_layout(np.ascontiguousarray(vT[:, rows_g0])),
            "xb_q": _sbuf_layout(np.ascontiguousarray(qT[:, rows_g1])),
            "xb_k": _sbuf_layout(np.ascontiguousarray(kT[:, rows_g1])),
            "xb_v": _sbuf_layout(np.ascontiguousarray(vT[:, rows_g1])),
            "wq": wlay(np.ascontiguousarray(wq[hrows].T)),
            "wk": wlay(np.ascontiguousarray(wk[hrows].T)),
            "wv": wlay(np.ascontiguousarray(wv[hrows].T)),
            "wo": np.ascontiguousarray(
                wov.reshape(3, 128, 1024).transpose(1, 0, 2).reshape(128, -1)),
            "ball": ballv,
        }
        in_maps.append(m)
    return in_maps


def _combine(results, bo):
    y = np.zeros((8192, 1024), np.float32)
    for c in range(8):
        sc, rc = c // 2, c // 4
        y[2048 * sc:2048 * (sc + 1)] += results[c]["y0"].astype(np.float32)
        y[4096 * rc + 1:4096 * (rc + 1):2] += results[c]["y1"].astype(np.float32)
    y += np.asarray(bo, np.float32)
    return y.reshape(1, 8192, 1024)


def kernel(query, key, value, Wq, bq, Wk, bk, Wv, bv, Wo, bo,
           _trace=False, _trace_cores=None):
    from concourse import bass_utils
    nc = _get_program()
    in_maps = _prep_inputs(query, key, value, Wq, bq, Wk, bk, Wv, bv, Wo, bo)
    res = bass_utils.run_bass_kernel_spmd(
        nc, in_maps, core_ids=list(range(8)),
        trace=_trace, trace_cores=_trace_cores)
    _CACHE["last_results"] = res
    return _combine(res.results, bo)


# revision 22
# speedup vs baseline: 1.2386x; 1.0100x over previous
"""BlockSparseRingMultiheadDilatedAttention Trainium2 kernel (v3).

Problem (hardcoded): B=1, N=8192, E=1024, H=16 heads, D=64.
Two dilated groups: g0 = heads 0-7, seg 2048, dilation 1;
                    g1 = heads 8-15, seg 4096, dilation 2, offset 1 (odd positions).
Causal within each (gathered) segment.

Sharding over 8 cores (uniform SPMD program, per-core data):
  core c: a = c%2, sc = c//2, b = c%4, rc = c//4
    g0: seg sc (rows 2048*sc .. +2048), heads 4a..4a+4   (4 blocks of [2048 x 2048])
    g1: seg rc odd rows (gathered, 2048 rows), heads 8+2b..+2 (2 blocks)
  Host pre-slices inputs (bf16 cast, odd-row gather, weight head slices,
  SBUF-layout rearrange) so the device program is identical on every core.
  Host sums the per-core partial output projections (disjoint head
  contributions, bf16) and adds bo.

v3 device dataflow:
  - few LARGE input DMAs (2 per x tensor, 1 per weight tensor) in final SBUF
    layout -- per-DMA fixed costs gated startup in v1/v2 (single HW queue,
    ~2us/DMA completion latency)
  - attention processes BOTH heads of a pair per chunk: two row-tiled
    (tile_position (0,0)/(64,0)) K=64 S-matmuls run concurrently in the PE
    array, land in one 2-bank PSUM tile -> ONE exp per [128,1024]
  - softmax: denominators from the appended-ones row of the PV matmul;
    numerators copied UNNORMALIZED to oT (releases PSUM fast); denominator
    rows gathered onto 8 partitions of one tile -> ONE batched DVE
    reciprocal per pair; broadcast+normalize muls run as fillers inside the
    NEXT pair's window (keeps the DVE FIFO from stalling PE -> HAM warm)
  - pair-2 qkv projections + V transposes are fillers inside pair 0/1's
    ACT(exp)-limited windows; y0/y1 out-proj chunks stream inside pair 2
  - outputs in bf16 (host combines in fp32)
"""

import numpy as np
import ml_dtypes
from collections import deque

BF16 = ml_dtypes.bfloat16

SEG = 2048          # rows per attention block (both groups, post-gather)
E = 1024            # embedding
NQ = 512            # tq chunk (one PSUM bank of fp32)
NTQ = SEG // NQ     # 4 tq chunks per block
NTK = SEG // 128    # 16 tk chunks per block
ECH = E // 128      # 8 embedding chunks

_CACHE = {}


def _build_program():
    import concourse.bacc as bacc
    import concourse.mybir as mybir
    import concourse.tile as tile

    dt = mybir.dt
    nc = bacc.Bacc("TRN2", target_bir_lowering=False, debug=False,
                   enable_asserts=False)

    # ---- DRAM I/O (uniform across cores; host slices per core) ----
    # x tensors already in SBUF layout [128, 2(pos half) * 8(ec) * 1024(pos)]
    xs = {}
    for sel in ("a", "b"):
        for inp in ("q", "k", "v"):
            xs[(sel, inp)] = nc.dram_tensor(
                f"x{sel}_{inp}", [128, 2 * ECH * 1024], dt.bfloat16,
                kind="ExternalInput").ap()
    ws = {inp: nc.dram_tensor(f"w{inp}", [128, 384 * ECH], dt.bfloat16,
                              kind="ExternalInput").ap()
          for inp in ("q", "k", "v")}
    wo = nc.dram_tensor("wo", [128, 3 * E], dt.bfloat16,
                        kind="ExternalInput").ap()
    ball = nc.dram_tensor("ball", [128, 9], dt.float32,
                          kind="ExternalInput").ap()
    y0 = nc.dram_tensor("y0", [SEG, E], dt.bfloat16, kind="ExternalOutput").ap()
    y1 = nc.dram_tensor("y1", [SEG, E], dt.bfloat16, kind="ExternalOutput").ap()

    with tile.TileContext(nc) as tc:
        from contextlib import ExitStack
        with ExitStack() as ctx:
            const = ctx.enter_context(tc.tile_pool(name="const", bufs=1))
            wpool = ctx.enter_context(tc.tile_pool(name="wpool", bufs=1))
            xpool = ctx.enter_context(tc.tile_pool(name="xpool", bufs=3))
            vtp = ctx.enter_context(tc.tile_pool(name="vtp", bufs=2))
            qkt = ctx.enter_context(tc.tile_pool(name="qkt", bufs=1))
            vnat = ctx.enter_context(tc.tile_pool(name="vnat", bufs=1))
            otp = ctx.enter_context(tc.tile_pool(name="otp", bufs=1))
            ptp = ctx.enter_context(tc.tile_pool(name="ptp", bufs=2))
            dnp = ctx.enter_context(tc.tile_pool(name="dnp", bufs=1))
            dsp_p = ctx.enter_context(tc.tile_pool(name="dsp_p", bufs=2))
            rcq = ctx.enter_context(tc.tile_pool(name="rcq", bufs=2))
            rbp = ctx.enter_context(tc.tile_pool(name="rbp", bufs=2))
            rbtp = ctx.enter_context(tc.tile_pool(name="rbtp", bufs=1))
            ysb = ctx.enter_context(tc.tile_pool(name="ysb", bufs=2))
            ps_s = ctx.enter_context(
                tc.tile_pool(name="ps_s", bufs=2, space="PSUM"))
            ps_b = ctx.enter_context(
                tc.tile_pool(name="ps_b", bufs=1, space="PSUM"))
            ps_o = ctx.enter_context(
                tc.tile_pool(name="ps_o", bufs=2, space="PSUM"))

            # ---- constants: identity (PE transpose), causal chunk masks ----
            ident = const.tile([128, 128], dt.bfloat16, tag="ident")
            nc.gpsimd.memset(ident, 1.0)
            nc.gpsimd.affine_select(
                out=ident, in_=ident, compare_op=mybir.AluOpType.is_equal,
                fill=0.0, base=0, pattern=[[-1, 128]], channel_multiplier=1)
            # additive causal mask for the 128-wide diagonal strip of a
            # diagonal k-chunk (both heads): 0 where f - p >= 0, else -1e9.
            # Applied to S in PSUM right after the S matmuls (concurrent
            # with the previous chunk's exp -> off the critical chain).
            maskadd = const.tile([128, 256], dt.bfloat16, tag="maskadd")
            nc.gpsimd.memset(maskadd, 0.0)
            nc.gpsimd.affine_select(
                out=maskadd.rearrange("p (hh f) -> p hh f", hh=2),
                in_=maskadd.rearrange("p (hh f) -> p hh f", hh=2),
                compare_op=mybir.AluOpType.is_ge,
                fill=-1e9, base=0, pattern=[[0, 2], [1, 128]],
                channel_multiplier=-1)

            # ---- weights: one DMA per tensor (host pre-layouts) ----
            wTa = {}
            for inp in ("q", "k", "v"):
                t = wpool.tile([128, 384 * ECH], dt.bfloat16,
                               tag=f"wT_{inp}", name=f"wT_{inp}")
                nc.sync.dma_start(out=t, in_=ws[inp])
                wTa[inp] = t
            wT = {inp: [wTa[inp].rearrange("p (ec x) -> p ec x", x=384)
                        [:, :, 128 * p:128 * (p + 1)]
                        for p in range(3)] for inp in ("q", "k", "v")}
            woTa = wpool.tile([128, 3 * E], dt.bfloat16, tag="woT")
            nc.sync.dma_start(out=woTa, in_=wo)
            woT = [woTa[:, E * p:E * (p + 1)] for p in range(3)]
            ballt = wpool.tile([128, 9], dt.float32, tag="ball")
            nc.sync.dma_start(out=ballt, in_=ball)
            bsb = {}
            for i, inp in enumerate(("q", "k", "v")):
                for p in range(3):
                    bsb[(inp, p)] = ballt[:, 3 * i + p:3 * i + p + 1]

            # ---- persistent per-pair activations ----
            qT = [qkt.tile([128, SEG], dt.bfloat16, tag=f"qT{p}", name=f"qT{p}")
                  for p in range(3)]
            kT = [qkt.tile([128, SEG], dt.bfloat16, tag=f"kT{p}", name=f"kT{p}")
                  for p in range(3)]
            vn = [vnat.tile([128, NTK * 130], dt.bfloat16, tag=f"vn{p}",
                            name=f"vn{p}")
                  for p in range(3)]
            for p in range(3):
                ones_view = vn[p].rearrange("p (k x) -> p k x", x=130)
                nc.gpsimd.memset(ones_view[:, :, 64:65], 1.0)
                nc.gpsimd.memset(ones_view[:, :, 129:130], 1.0)
            oT = [otp.tile([128, SEG], dt.bfloat16, tag=f"oT{p}", name=f"oT{p}")
                  for p in range(3)]

            # ---- input loads: 2 DMAs (position halves) per tensor ----
            xt_tiles = {}
            xt_raw = {}
            HC = ECH * 1024     # columns per position half
            for sel, inp in (("a", "q"), ("a", "k"), ("a", "v"),
                             ("b", "q"), ("b", "k"), ("b", "v")):
                t = xpool.tile([128, 2 * HC], dt.bfloat16, tag="xt")
                xt_raw[(sel, inp)] = t
                xt_tiles[(sel, inp)] = t.rearrange(
                    "p (t2 ec s) -> p t2 ec s", t2=2, ec=ECH)

            def load_x_half(sel, inp, t2):
                t = xt_raw[(sel, inp)]
                nc.sync.dma_start(
                    out=t[:, HC * t2:HC * (t2 + 1)],
                    in_=xs[(sel, inp)][:, HC * t2:HC * (t2 + 1)])
            # xa halves interleaved (t2=0 of q,k,v first) so attention can
            # start after the first position-half is projected; xb after
            for t2 in range(2):
                for inp in ("q", "k", "v"):
                    load_x_half("a", inp, t2)
            for inp in ("q", "k", "v"):
                for t2 in range(2):
                    load_x_half("b", inp, t2)

            # ---- projection building blocks ----
            def proj_half(acc, inp, p, t2, half, sel):
                xt = xt_tiles[(sel, inp)]
                for ec in range(ECH):
                    nc.tensor.matmul(
                        acc[:, NQ * half:NQ * (half + 1)],
                        wT[inp][p][:, ec, :],
                        xt[:, t2, ec, NQ * half:NQ * (half + 1)],
                        start=(ec == 0), stop=(ec == ECH - 1))

            def vtrans_batch(p, i0, vtile):
                # transpose 4 [128,128] chunks of vT into V-natural slices
                ptr = ps_b.tile([128, 4 * 128], dt.bfloat16, tag="b",
                                name="ptr")
                for qq in range(4):
                    nc.tensor.transpose(
                        ptr[:, 128 * qq:128 * (qq + 1)],
                        vtile[:, 128 * (i0 + qq):128 * (i0 + qq + 1)], ident)
                src = ptr.rearrange("p (c h d) -> p c h d", c=4, h=2)
                dst = vn[p][:, 130 * i0:130 * (i0 + 4)].rearrange(
                    "p (c h x) -> p c h x", c=4, x=65)[:, :, :, 0:64]
                nc.vector.tensor_copy(dst, src)

            # ---- proj group: one [128,1024] output col-block ----
            def proj_group(inp, p, t2, dst, sel, act_bias):
                acc = (ps_s if act_bias else ps_b).tile(
                    [128, 2 * NQ], dt.float32, tag=("s" if act_bias else "b"),
                    name="proj")
                proj_half(acc, inp, p, t2, 0, sel)
                proj_half(acc, inp, p, t2, 1, sel)
                if act_bias:
                    nc.scalar.activation(
                        dst[:, 2 * NQ * t2:2 * NQ * (t2 + 1)], acc,
                        mybir.ActivationFunctionType.Identity,
                        bias=bsb[(inp, p)], scale=1.0)
                else:
                    nc.vector.tensor_scalar_add(
                        dst[:, 2 * NQ * t2:2 * NQ * (t2 + 1)], acc,
                        bsb[(inp, p)])

            # ---- Phase A: t2=0 projections for pairs 0,1 (pre-attention) --
            vts = {}
            for inp in ("q", "k", "v"):
                for p in (0, 1):
                    if inp == "v":
                        dst = vtp.tile([128, SEG], dt.bfloat16, tag="vT",
                                       name="vT")
                        vts[p] = dst
                    else:
                        dst = (qT if inp == "q" else kT)[p]
                    proj_group(inp, p, 0, dst, "a", True)
                    if inp == "v":
                        vtrans_batch(p, 0, dst)
                        vtrans_batch(p, 4, dst)

            # ---- filler machinery (global slot counter, absolute gates) --
            fillers = deque()   # (min_slot, thunk)
            slot_counter = [0]

            def slot_cb():
                slot_counter[0] += 1
                if fillers and fillers[0][0] <= slot_counter[0]:
                    fillers.popleft()[1]()

            def drain_fillers():
                while fillers:
                    fillers.popleft()[1]()

            # t2=1 projections for pairs 0,1 -> early attn(0) fillers
            for gi, (inp, p) in enumerate(
                    (("q", 0), ("q", 1), ("k", 0), ("k", 1),
                     ("v", 0), ("v", 1))):
                dst = vts[p] if inp == "v" else (qT if inp == "q" else kT)[p]
                fillers.append(
                    (2 + gi, lambda inp=inp, p=p, dst=dst:
                     proj_group(inp, p, 1, dst, "a", True)))
            for gi, (p, i0) in enumerate(((0, 8), (0, 12), (1, 8), (1, 12))):
                fillers.append(
                    (9 + gi, lambda p=p, i0=i0: vtrans_batch(p, i0, vts[p])))

            # pair-2 projections (xb) as later attn(0) fillers
            def make_projB_fillers():
                vtile = vtp.tile([128, SEG], dt.bfloat16, tag="vT",
                                 name="vtB")
                gates = {"q": 18, "k": 26, "v": 34}
                for inp in ("q", "k", "v"):
                    dst = {"q": qT[2], "k": kT[2], "v": vtile}[inp]
                    g = gates[inp]
                    for t2 in range(2):
                        fillers.append(
                            (g + 2 * t2, lambda inp=inp, t2=t2, dst=dst:
                             proj_group(inp, 2, t2, dst, "b", False)))
                for bi, i0 in enumerate((0, 4, 8, 12)):
                    fillers.append(
                        (38 + bi, lambda i0=i0: vtrans_batch(2, i0, vtile)))

            # out-projection chunk fillers
            def y_unit(ydram, pairs, m, cast_on_act=False):
                def th():
                    accy = ps_b.tile([128, 2 * NQ], dt.float32, tag="b",
                                     name="accy")
                    for jc in range(2):
                        for idx, p in enumerate(pairs):
                            nc.tensor.matmul(
                                accy[:, NQ * jc:NQ * (jc + 1)],
                                oT[p][:, 128 * m:128 * (m + 1)],
                                woT[p][:, NQ * jc:NQ * (jc + 1)],
                                start=(idx == 0), stop=(idx == len(pairs) - 1))
                    t = ysb.tile([128, 2 * NQ], dt.bfloat16, tag="ysb")
                    if cast_on_act:
                        nc.scalar.copy(t, accy)
                    else:
                        nc.vector.tensor_copy(t, accy)
                    nc.sync.dma_start(
                        out=ydram[128 * m:128 * (m + 1), :], in_=t)
                return th

            # ---- normalization (DMA-scatter batched reciprocal) ----
            def drow(j, h):
                return 32 * (2 * (j % 2) + h), NQ * (j // 2)

            dens = [dnp.tile([128, 2 * NQ], dt.bfloat16, tag="dens",
                             name=f"dens{p}") for p in range(3)]
            dsp = [dsp_p.tile([32, 8 * 16], dt.bfloat16, tag="dsp",
                              name=f"dsp{p}") for p in range(3)]
            rsp = [dsp_p.tile([32, 8 * 16], dt.float32, tag="rsp",
                              name=f"rsp{p}") for p in range(3)]

            def recip_batch(p, us):
                # spread each den row over 32 partitions (64B DMA lines) so
                # ONE short-free-dim reciprocal covers the batch
                def th():
                    for u in us:
                        r, c = drow(u // 2, u % 2)
                        nc.sync.dma_start(
                            out=dsp[p][:, 16 * u:16 * (u + 1)],
                            in_=dens[p][r:r + 1, c:c + NQ])
                    u0, u1 = us[0], us[-1] + 1
                    nc.vector.reciprocal(rsp[p][:, 16 * u0:16 * u1],
                                         dsp[p][:, 16 * u0:16 * u1])
                return th

            def norm_bc_unit(p, j, rb2s):
                def th():
                    rb2 = rbp.tile([128, NQ], dt.float32, tag="rb")
                    rb2s[j] = rb2
                    for h in range(2):
                        u = 2 * j + h
                        rc = rcq.tile([1, NQ], dt.float32, tag="rc")
                        nc.sync.dma_start(out=rc,
                                          in_=rsp[p][:, 16 * u:16 * (u + 1)])
                        if h == 0:
                            nc.gpsimd.partition_broadcast(rb2[0:64, :], rc)
                        else:
                            rbt = rbtp.tile([64, NQ], dt.float32,
                                            tag="rbt")
                            nc.gpsimd.partition_broadcast(rbt, rc)
                            nc.sync.dma_start(out=rb2[64:128, :], in_=rbt)
                return th

            def norm_mul_unit(p, j, rb2s):
                def th():
                    sl = oT[p][:, NQ * j:NQ * (j + 1)]
                    nc.vector.tensor_mul(sl, sl, rb2s.pop(j))
                return th

            def norm_fillers(p, base, js):
                rb2s = {}
                ents = [(base, recip_batch(p, [u for j in js
                                               for u in (2 * j, 2 * j + 1)]))]
                for i, j in enumerate(js):
                    ents.append((base + 2 + i, norm_bc_unit(p, j, rb2s)))
                    ents.append((base + 4 + i, norm_mul_unit(p, j, rb2s)))
                ents.sort(key=lambda e: e[0])
                return ents

            # ---- attention: flat chunk stream with cross-boundary S
            # lookahead (keeps ACT fed across j/pair boundaries) ----
            def s_emit(p, j, ci):
                s = ps_s.tile([128, 2 * NQ], dt.float32, tag="s", name="s")
                nc.tensor.matmul(
                    s[:, 0:NQ],
                    kT[p][0:64, 128 * ci:128 * (ci + 1)],
                    qT[p][0:64, NQ * j:NQ * (j + 1)],
                    start=True, stop=True)
                nc.tensor.matmul(
                    s[:, NQ:2 * NQ],
                    kT[p][64:128, 128 * ci:128 * (ci + 1)],
                    qT[p][64:128, NQ * j:NQ * (j + 1)],
                    start=True, stop=True)
                di = ci - 4 * j
                if di >= 0:
                    sv = s.rearrange("p (h x) -> p h x", h=2)[
                        :, :, 128 * di:128 * (di + 1)]
                    nc.vector.tensor_add(
                        sv, sv,
                        maskadd.rearrange("p (h x) -> p h x", h=2))
                return s

            chunks = []         # flat stream: (p, j, ci, nchunks)
            hooks = {}          # (p, j) -> after_j hook
            for p in range(3):
                for j in range(NTQ):
                    n = 4 * (j + 1)
                    for ci in range(n):
                        chunks.append((p, j, ci, n))

            def attn_all():
                accs = {}
                s_tiles = {}
                first = chunks[0]
                s_tiles[first[:3]] = s_emit(*first[:3])
                for idx, (p, j, ci, n) in enumerate(chunks):
                    if ci == 0:
                        acc0 = ps_o.tile([128, NQ], dt.float32, tag="acc",
                                         name="acc0")
                        acc1 = ps_o.tile([128, NQ], dt.float32, tag="acc",
                                         name="acc1")
                        accs[(p, j)] = (acc0, acc1)
                    acc0, acc1 = accs[(p, j)]
                    s = s_tiles.pop((p, j, ci))
                    pt = ptp.tile([128, 2 * NQ], dt.bfloat16, tag="pt",
                                  name="pt")
                    # cols [0, 128*di) of a diagonal chunk are fully masked:
                    # exp and the PV matmuls skip them entirely
                    c0 = max(ci - 4 * j, 0) * 128
                    if c0 == 0:
                        nc.scalar.activation(
                            pt, s, mybir.ActivationFunctionType.Exp,
                            bias=0.0, scale=0.125)
                    else:
                        pv = pt.rearrange("p (h x) -> p h x", h=2)[
                            :, :, c0:NQ]
                        sv = s.rearrange("p (h x) -> p h x", h=2)[
                            :, :, c0:NQ]
                        nc.scalar.activation(
                            pv, sv, mybir.ActivationFunctionType.Exp,
                            bias=0.0, scale=0.125)
                    if idx + 1 < len(chunks):
                        nxt = chunks[idx + 1]
                        s_tiles[nxt[:3]] = s_emit(*nxt[:3])
                    last = (ci == n - 1)
                    nc.tensor.matmul(
                        acc0[0:65, c0:NQ],
                        vn[p][:, 130 * ci:130 * ci + 65],
                        pt[:, c0:NQ], start=(ci == 0), stop=last)
                    nc.tensor.matmul(
                        acc1[0:65, c0:NQ],
                        vn[p][:, 130 * ci + 65:130 * ci + 130],
                        pt[:, NQ + c0:2 * NQ], start=(ci == 0), stop=last)
                    slot_cb()
                    if last:
                        for h, acc in enumerate((acc0, acc1)):
                            nc.vector.tensor_copy(
                                oT[p][64 * h:64 * h + 64,
                                      NQ * j:NQ * (j + 1)],
                                acc[0:64, :])
                            r, c = drow(j, h)
                            nc.vector.tensor_copy(
                                dens[p][r:r + 1, c:c + NQ], acc[64:65, :])
                        accs.pop((p, j))
                        hk = hooks.get((p, j))
                        if hk is not None:
                            hk()

            # norm/y filler schedule (absolute slots: pair p spans
            # [40p, 40p+40))
            def queue_pair0_norm():
                fillers.extend(norm_fillers(0, 40, [0, 1, 2, 3]))
            hooks[(0, 3)] = queue_pair0_norm

            def queue_pair1_norm():
                fillers.extend(norm_fillers(1, 80, [0, 1, 2, 3]))
                for m in range(16):
                    fillers.append((89 + m, y_unit(y0, (0, 1), m)))
            hooks[(1, 3)] = queue_pair1_norm

            def queue_pair2_mid():
                fillers.extend(norm_fillers(2, 105, [0, 1, 2]))
                for m in range(12):
                    fillers.append((113 + m, y_unit(y1, (2,), m)))
            hooks[(2, 2)] = queue_pair2_mid

            make_projB_fillers()
            attn_all()
            drain_fillers()
            # tail: j3 norm + last y1 units
            rb2s_t = {}
            recip_batch(2, [6, 7])()
            norm_bc_unit(2, 3, rb2s_t)()
            norm_mul_unit(2, 3, rb2s_t)()
            for m in range(12, 16):
                y_unit(y1, (2,), m, cast_on_act=True)()

    nc.compile()
    return nc


def _get_program():
    if "nc" not in _CACHE:
        _CACHE["nc"] = _build_program()
    return _CACHE["nc"]


def _sbuf_layout(xT):
    """[1024, 2048] (e, pos) -> [128, 2*8*1024]: pos-half major, ec, pos."""
    # [ec, 128, t2, 1024] -> [128, t2, ec, 1024]
    return np.ascontiguousarray(
        xT.reshape(ECH, 128, 2, 1024).transpose(1, 2, 0, 3).reshape(
            128, 2 * ECH * 1024))


def _prep_inputs(query, key, value, Wq, bq, Wk, bk, Wv, bv, Wo, bo):
    """Build the 8 per-core input maps (host-side slicing + bf16 cast)."""
    q = np.asarray(query, np.float32).reshape(8192, 1024).astype(BF16)
    k = np.asarray(key, np.float32).reshape(8192, 1024).astype(BF16)
    v = np.asarray(value, np.float32).reshape(8192, 1024).astype(BF16)
    wq = np.asarray(Wq, np.float32).astype(BF16)
    wk = np.asarray(Wk, np.float32).astype(BF16)
    wv = np.asarray(Wv, np.float32).astype(BF16)
    wo_f = np.asarray(Wo, np.float32).astype(BF16)
    bqf = np.asarray(bq, np.float32)
    bkf = np.asarray(bk, np.float32)
    bvf = np.asarray(bv, np.float32)

    qT, kT, vT = q.T, k.T, v.T  # [1024, 8192] views
    in_maps = []
    for c in range(8):
        a, sc, b, rc = c % 2, c // 2, c % 4, c // 4
        rows_g0 = slice(2048 * sc, 2048 * (sc + 1))
        rows_g1 = slice(4096 * rc + 1, 4096 * (rc + 1), 2)
        hrows = np.r_[256 * a:256 * a + 256, 512 + 128 * b:512 + 128 * b + 128]

        def wlay(w):
            # [1024, 384] -> [128, 8*384] (ec-blocked)
            return np.ascontiguousarray(
                w.reshape(ECH, 128, 384).transpose(1, 0, 2).reshape(128, -1))

        wov = np.ascontiguousarray(wo_f[:, hrows].T)  # [384, 1024]
        ballv = np.stack([np.ascontiguousarray(bf[hrows]).reshape(3, 128)
                          for bf in (bqf, bkf, bvf)], axis=0)  # [3,3,128]
        # ball[128, 9]: col 3i+p = input i, pair p
        ballv = np.ascontiguousarray(ballv.reshape(9, 128).T)
        m = {
            "xa_q": _sbuf_layout(np.ascontiguousarray(qT[:, rows_g0])),
            "xa_k": _sbuf_layout(np.ascontiguousarray(kT[:, rows_g0])),
            "xa_v": _sbuf_layout(np.ascontiguousarray(vT[:, rows_g0])),
            "xb_q": _sbuf_layout(np.ascontiguousarray(qT[:, rows_g1])),
            "xb_k": _sbuf_layout(np.ascontiguousarray(kT[:, rows_g1])),
            "xb_v": _sbuf_layout(np.ascontiguousarray(vT[:, rows_g1])),
            "wq": wlay(np.ascontiguousarray(wq[hrows].T)),
            "wk": wlay(np.ascontiguousarray(wk[hrows].T)),
            "wv": wlay(np.ascontiguousarray(wv[hrows].T)),
            "wo": np.ascontiguousarray(
                wov.reshape(3, 128, 1024).transpose(1, 0, 2).reshape(128, -1)),
            "ball": ballv,
        }
        in_maps.append(m)
    return in_maps


def _combine(results, bo):
    y = np.zeros((8192, 1024), np.float32)
    for c in range(8):
        sc, rc = c // 2, c // 4
        y[2048 * sc:2048 * (sc + 1)] += results[c]["y0"].astype(np.float32)
        y[4096 * rc + 1:4096 * (rc + 1):2] += results[c]["y1"].astype(np.float32)
    y += np.asarray(bo, np.float32)
    return y.reshape(1, 8192, 1024)


def kernel(query, key, value, Wq, bq, Wk, bk, Wv, bv, Wo, bo,
           _trace=False, _trace_cores=None):
    from concourse import bass_utils
    nc = _get_program()
    in_maps = _prep_inputs(query, key, value, Wq, bq, Wk, bk, Wv, bv, Wo, bo)
    res = bass_utils.run_bass_kernel_spmd(
        nc, in_maps, core_ids=list(range(8)),
        trace=_trace, trace_cores=_trace_cores)
    _CACHE["last_results"] = res
    return _combine(res.results, bo)


# revision 24
# speedup vs baseline: 1.2762x; 1.0303x over previous
"""BlockSparseRingMultiheadDilatedAttention Trainium2 kernel (v3).

Problem (hardcoded): B=1, N=8192, E=1024, H=16 heads, D=64.
Two dilated groups: g0 = heads 0-7, seg 2048, dilation 1;
                    g1 = heads 8-15, seg 4096, dilation 2, offset 1 (odd positions).
Causal within each (gathered) segment.

Sharding over 8 cores (uniform SPMD program, per-core data):
  core c: a = c%2, sc = c//2, b = c%4, rc = c//4
    g0: seg sc (rows 2048*sc .. +2048), heads 4a..4a+4   (4 blocks of [2048 x 2048])
    g1: seg rc odd rows (gathered, 2048 rows), heads 8+2b..+2 (2 blocks)
  Host pre-slices inputs (bf16 cast, odd-row gather, weight head slices,
  SBUF-layout rearrange) so the device program is identical on every core.
  Host sums the per-core partial output projections (disjoint head
  contributions, bf16) and adds bo.

v3 device dataflow:
  - few LARGE input DMAs (2 per x tensor, 1 per weight tensor) in final SBUF
    layout -- per-DMA fixed costs gated startup in v1/v2 (single HW queue,
    ~2us/DMA completion latency)
  - attention processes BOTH heads of a pair per chunk: two row-tiled
    (tile_position (0,0)/(64,0)) K=64 S-matmuls run concurrently in the PE
    array, land in one 2-bank PSUM tile -> ONE exp per [128,1024]
  - softmax: denominators from the appended-ones row of the PV matmul;
    numerators copied UNNORMALIZED to oT (releases PSUM fast); denominator
    rows gathered onto 8 partitions of one tile -> ONE batched DVE
    reciprocal per pair; broadcast+normalize muls run as fillers inside the
    NEXT pair's window (keeps the DVE FIFO from stalling PE -> HAM warm)
  - pair-2 qkv projections + V transposes are fillers inside pair 0/1's
    ACT(exp)-limited windows; y0/y1 out-proj chunks stream inside pair 2
  - outputs in bf16 (host combines in fp32)
"""

import numpy as np
import ml_dtypes
from collections import deque

BF16 = ml_dtypes.bfloat16

SEG = 2048          # rows per attention block (both groups, post-gather)
E = 1024            # embedding
NQ = 512            # tq chunk (one PSUM bank of fp32)
NTQ = SEG // NQ     # 4 tq chunks per block
NTK = SEG // 128    # 16 tk chunks per block
ECH = E // 128      # 8 embedding chunks

_CACHE = {}


def _build_program():
    import concourse.bacc as bacc
    import concourse.mybir as mybir
    import concourse.tile as tile

    dt = mybir.dt
    nc = bacc.Bacc("TRN2", target_bir_lowering=False, debug=False,
                   enable_asserts=False)

    # ---- DRAM I/O (uniform across cores; host slices per core) ----
    # x tensors already in SBUF layout [128, 2(pos half) * 8(ec) * 1024(pos)]
    xs = {}
    for sel in ("a", "b"):
        for inp in ("q", "k", "v"):
            xs[(sel, inp)] = nc.dram_tensor(
                f"x{sel}_{inp}", [128, 2 * ECH * 1024], dt.bfloat16,
                kind="ExternalInput").ap()
    ws = {inp: nc.dram_tensor(f"w{inp}", [128, 384 * ECH], dt.bfloat16,
                              kind="ExternalInput").ap()
          for inp in ("q", "k", "v")}
    wo = nc.dram_tensor("wo", [128, 3 * E], dt.bfloat16,
                        kind="ExternalInput").ap()
    ball = nc.dram_tensor("ball", [128, 9], dt.float32,
                          kind="ExternalInput").ap()
    y0 = nc.dram_tensor("y0", [SEG, E], dt.bfloat16, kind="ExternalOutput").ap()
    y1 = nc.dram_tensor("y1", [SEG, E], dt.bfloat16, kind="ExternalOutput").ap()

    with tile.TileContext(nc) as tc:
        from contextlib import ExitStack
        with ExitStack() as ctx:
            const = ctx.enter_context(tc.tile_pool(name="const", bufs=1))
            wpool = ctx.enter_context(tc.tile_pool(name="wpool", bufs=1))
            xpool = ctx.enter_context(tc.tile_pool(name="xpool", bufs=4))
            vtp = ctx.enter_context(tc.tile_pool(name="vtp", bufs=2))
            qkt = ctx.enter_context(tc.tile_pool(name="qkt", bufs=1))
            vnat = ctx.enter_context(tc.tile_pool(name="vnat", bufs=1))
            otp = ctx.enter_context(tc.tile_pool(name="otp", bufs=1))
            ptp = ctx.enter_context(tc.tile_pool(name="ptp", bufs=3))
            dnp = ctx.enter_context(tc.tile_pool(name="dnp", bufs=1))
            dsp_p = ctx.enter_context(tc.tile_pool(name="dsp_p", bufs=1))
            rcq = ctx.enter_context(tc.tile_pool(name="rcq", bufs=2))
            rbp = ctx.enter_context(tc.tile_pool(name="rbp", bufs=2))
            rbtp = ctx.enter_context(tc.tile_pool(name="rbtp", bufs=1))
            ysb = ctx.enter_context(tc.tile_pool(name="ysb", bufs=2))
            ps_s = ctx.enter_context(
                tc.tile_pool(name="ps_s", bufs=2, space="PSUM"))
            ps_b = ctx.enter_context(
                tc.tile_pool(name="ps_b", bufs=1, space="PSUM"))
            ps_o = ctx.enter_context(
                tc.tile_pool(name="ps_o", bufs=2, space="PSUM"))

            # ---- constants: identity (PE transpose), causal chunk masks ----
            ident = const.tile([128, 128], dt.bfloat16, tag="ident")
            nc.gpsimd.memset(ident, 1.0)
            nc.gpsimd.affine_select(
                out=ident, in_=ident, compare_op=mybir.AluOpType.is_equal,
                fill=0.0, base=0, pattern=[[-1, 128]], channel_multiplier=1)
            # additive causal mask for the 128-wide diagonal strip of a
            # diagonal k-chunk (both heads): 0 where f - p >= 0, else -1e9.
            # Applied to S in PSUM right after the S matmuls (concurrent
            # with the previous chunk's exp -> off the critical chain).
            maskadd = const.tile([128, 256], dt.bfloat16, tag="maskadd")
            nc.gpsimd.memset(maskadd, 0.0)
            nc.gpsimd.affine_select(
                out=maskadd.rearrange("p (hh f) -> p hh f", hh=2),
                in_=maskadd.rearrange("p (hh f) -> p hh f", hh=2),
                compare_op=mybir.AluOpType.is_ge,
                fill=-1e9, base=0, pattern=[[0, 2], [1, 128]],
                channel_multiplier=-1)

            # ---- weights: one DMA per tensor (host pre-layouts) ----
            wTa = {}
            for inp in ("q", "k", "v"):
                t = wpool.tile([128, 384 * ECH], dt.bfloat16,
                               tag=f"wT_{inp}", name=f"wT_{inp}")
                nc.sync.dma_start(out=t, in_=ws[inp])
                wTa[inp] = t
            wT = {inp: [wTa[inp].rearrange("p (ec x) -> p ec x", x=384)
                        [:, :, 128 * p:128 * (p + 1)]
                        for p in range(3)] for inp in ("q", "k", "v")}
            woTa = wpool.tile([128, 3 * E], dt.bfloat16, tag="woT")
            nc.sync.dma_start(out=woTa, in_=wo)
            woT = [woTa[:, E * p:E * (p + 1)] for p in range(3)]
            ballt = wpool.tile([128, 9], dt.float32, tag="ball")
            nc.sync.dma_start(out=ballt, in_=ball)
            bsb = {}
            for i, inp in enumerate(("q", "k", "v")):
                for p in range(3):
                    bsb[(inp, p)] = ballt[:, 3 * i + p:3 * i + p + 1]

            # ---- persistent per-pair activations ----
            qT = [qkt.tile([128, SEG], dt.bfloat16, tag=f"qT{p}", name=f"qT{p}")
                  for p in range(3)]
            kT = [qkt.tile([128, SEG], dt.bfloat16, tag=f"kT{p}", name=f"kT{p}")
                  for p in range(3)]
            vn = [vnat.tile([128, NTK * 130], dt.bfloat16, tag=f"vn{p}",
                            name=f"vn{p}")
                  for p in range(3)]
            for p in range(3):
                ones_view = vn[p].rearrange("p (k x) -> p k x", x=130)
                nc.gpsimd.memset(ones_view[:, :, 64:65], 1.0)
                nc.gpsimd.memset(ones_view[:, :, 129:130], 1.0)
            oT = [otp.tile([128, SEG], dt.bfloat16, tag=f"oT{p}", name=f"oT{p}")
                  for p in range(3)]

            # ---- input loads: 2 DMAs (position halves) per tensor ----
            xt_tiles = {}
            HC = ECH * 1024     # columns per position half

            def load_x_half(sel, inp, t2):
                t = xpool.tile([128, HC], dt.bfloat16, tag="xt", name="xt")
                nc.sync.dma_start(
                    out=t, in_=xs[(sel, inp)][:, HC * t2:HC * (t2 + 1)])
                xt_tiles[(sel, inp, t2)] = t.rearrange(
                    "p (ec s) -> p ec s", ec=ECH)
            # xa halves interleaved (t2=0 of q,k,v first) so attention can
            # start after the first position-half is projected; xb after
            for t2 in range(2):
                for inp in ("q", "k", "v"):
                    load_x_half("a", inp, t2)
            for inp in ("q", "k", "v"):
                for t2 in range(2):
                    load_x_half("b", inp, t2)

            # ---- projection building blocks ----
            def proj_half(acc, inp, p, t2, half, sel):
                xt = xt_tiles[(sel, inp, t2)]
                for ec in range(ECH):
                    nc.tensor.matmul(
                        acc[:, NQ * half:NQ * (half + 1)],
                        wT[inp][p][:, ec, :],
                        xt[:, ec, NQ * half:NQ * (half + 1)],
                        start=(ec == 0), stop=(ec == ECH - 1))

            def vtrans_batch(p, i0, vtile):
                # transpose 4 [128,128] chunks of vT into V-natural slices
                ptr = ps_b.tile([128, 4 * 128], dt.bfloat16, tag="b",
                                name="ptr")
                for qq in range(4):
                    nc.tensor.transpose(
                        ptr[:, 128 * qq:128 * (qq + 1)],
                        vtile[:, 128 * (i0 + qq):128 * (i0 + qq + 1)], ident)
                src = ptr.rearrange("p (c h d) -> p c h d", c=4, h=2)
                dst = vn[p][:, 130 * i0:130 * (i0 + 4)].rearrange(
                    "p (c h x) -> p c h x", c=4, x=65)[:, :, :, 0:64]
                nc.vector.tensor_copy(dst, src)

            # ---- proj group: one [128,1024] output col-block ----
            def proj_group(inp, p, t2, dst, sel, act_bias):
                acc = (ps_s if act_bias else ps_b).tile(
                    [128, 2 * NQ], dt.float32, tag=("s" if act_bias else "b"),
                    name="proj")
                proj_half(acc, inp, p, t2, 0, sel)
                proj_half(acc, inp, p, t2, 1, sel)
                if act_bias:
                    nc.scalar.activation(
                        dst[:, 2 * NQ * t2:2 * NQ * (t2 + 1)], acc,
                        mybir.ActivationFunctionType.Identity,
                        bias=bsb[(inp, p)], scale=1.0)
                else:
                    nc.vector.tensor_scalar_add(
                        dst[:, 2 * NQ * t2:2 * NQ * (t2 + 1)], acc,
                        bsb[(inp, p)])

            # ---- Phase A: t2=0 projections for pairs 0,1 (pre-attention) --
            vts = {}
            for inp in ("q", "k", "v"):
                for p in (0, 1):
                    if inp == "v":
                        dst = vtp.tile([128, SEG], dt.bfloat16, tag="vT",
                                       name="vT")
                        vts[p] = dst
                    else:
                        dst = (qT if inp == "q" else kT)[p]
                    proj_group(inp, p, 0, dst, "a", True)
                    if inp == "v":
                        vtrans_batch(p, 0, dst)
                        vtrans_batch(p, 4, dst)

            # ---- filler machinery (global slot counter, absolute gates) --
            fillers = deque()   # (min_slot, thunk)
            slot_counter = [0]

            def slot_cb():
                slot_counter[0] += 1
                if fillers and fillers[0][0] <= slot_counter[0]:
                    fillers.popleft()[1]()

            def drain_fillers():
                while fillers:
                    fillers.popleft()[1]()

            # t2=1 projections for pairs 0,1 -> early attn(0) fillers
            for gi, (inp, p) in enumerate(
                    (("q", 0), ("q", 1), ("k", 0), ("k", 1),
                     ("v", 0), ("v", 1))):
                dst = vts[p] if inp == "v" else (qT if inp == "q" else kT)[p]
                fillers.append(
                    (2 + gi, lambda inp=inp, p=p, dst=dst:
                     proj_group(inp, p, 1, dst, "a", True)))
            for gi, (p, i0) in enumerate(((0, 8), (0, 12), (1, 8), (1, 12))):
                fillers.append(
                    (9 + gi, lambda p=p, i0=i0: vtrans_batch(p, i0, vts[p])))

            # pair-2 projections (xb) as later attn(0) fillers
            def make_projB_fillers():
                vtile = vtp.tile([128, SEG], dt.bfloat16, tag="vT",
                                 name="vtB")
                gates = {"q": 14, "k": 22, "v": 41}
                for inp in ("q", "k", "v"):
                    dst = {"q": qT[2], "k": kT[2], "v": vtile}[inp]
                    g = gates[inp]
                    for t2 in range(2):
                        fillers.append(
                            (g + 2 * t2, lambda inp=inp, t2=t2, dst=dst:
                             proj_group(inp, 2, t2, dst, "b", False)))
                for bi, i0 in enumerate((0, 4, 8, 12)):
                    fillers.append(
                        (45 + bi, lambda i0=i0: vtrans_batch(2, i0, vtile)))

            # out-projection chunk fillers
            def y_unit(ydram, pairs, m, cast_on_act=False):
                def th():
                    accy = ps_b.tile([128, 2 * NQ], dt.float32, tag="b",
                                     name="accy")
                    for jc in range(2):
                        for idx, p in enumerate(pairs):
                            nc.tensor.matmul(
                                accy[:, NQ * jc:NQ * (jc + 1)],
                                oT[p][:, 128 * m:128 * (m + 1)],
                                woT[p][:, NQ * jc:NQ * (jc + 1)],
                                start=(idx == 0), stop=(idx == len(pairs) - 1))
                    t = ysb.tile([128, 2 * NQ], dt.bfloat16, tag="ysb")
                    if cast_on_act:
                        nc.scalar.copy(t, accy)
                    else:
                        nc.vector.tensor_copy(t, accy)
                    nc.sync.dma_start(
                        out=ydram[128 * m:128 * (m + 1), :], in_=t)
                return th

            # ---- normalization (DMA-scatter batched reciprocal) ----
            def drow(j, h):
                return 32 * (2 * (j % 2) + h), NQ * (j // 2)

            dens = [dnp.tile([128, 2 * NQ], dt.bfloat16, tag="dens",
                             name=f"dens{p}") for p in range(3)]
            dsp = [dsp_p.tile([32, 8 * 16], dt.bfloat16, tag="dsp",
                              name=f"dsp{p}") for p in range(3)]
            rsp = [dsp_p.tile([32, 8 * 16], dt.float32, tag="rsp",
                              name=f"rsp{p}") for p in range(3)]

            def recip_batch(p, us):
                # spread each den row over 32 partitions (64B DMA lines) so
                # ONE short-free-dim reciprocal covers the batch
                def th():
                    for u in us:
                        r, c = drow(u // 2, u % 2)
                        nc.gpsimd.dma_start(
                            out=dsp[p][:, 16 * u:16 * (u + 1)],
                            in_=dens[p][r:r + 1, c:c + NQ])
                    u0, u1 = us[0], us[-1] + 1
                    nc.vector.reciprocal(rsp[p][:, 16 * u0:16 * u1],
                                         dsp[p][:, 16 * u0:16 * u1])
                return th

            def norm_bc_unit(p, j, rb2s):
                def th():
                    rb2 = rbp.tile([128, NQ], dt.float32, tag="rb")
                    rb2s[j] = rb2
                    for h in range(2):
                        u = 2 * j + h
                        rc = rcq.tile([1, NQ], dt.float32, tag="rc")
                        nc.gpsimd.dma_start(
                            out=rc, in_=rsp[p][:, 16 * u:16 * (u + 1)])
                        if h == 0:
                            nc.gpsimd.partition_broadcast(rb2[0:64, :], rc)
                        else:
                            rbt = rbtp.tile([64, NQ], dt.float32,
                                            tag="rbt")
                            nc.gpsimd.partition_broadcast(rbt, rc)
                            nc.gpsimd.dma_start(out=rb2[64:128, :], in_=rbt)
                return th

            def norm_mul_unit(p, j, rb2s):
                def th():
                    sl = oT[p][:, NQ * j:NQ * (j + 1)]
                    nc.vector.tensor_mul(sl, sl, rb2s.pop(j))
                return th

            def norm_fillers(p, base, js):
                rb2s = {}
                ents = [(base, recip_batch(p, [u for j in js
                                               for u in (2 * j, 2 * j + 1)]))]
                for i, j in enumerate(js):
                    ents.append((base + 2 + i, norm_bc_unit(p, j, rb2s)))
                    ents.append((base + 4 + i, norm_mul_unit(p, j, rb2s)))
                ents.sort(key=lambda e: e[0])
                return ents

            # ---- attention: flat chunk stream with cross-boundary S
            # lookahead (keeps ACT fed across j/pair boundaries) ----
            def s_emit(p, j, ci):
                s = ps_s.tile([128, 2 * NQ], dt.float32, tag="s", name="s")
                c0 = max(ci - 4 * j, 0) * 128
                nc.tensor.matmul(
                    s[:, c0:NQ],
                    kT[p][0:64, 128 * ci:128 * (ci + 1)],
                    qT[p][0:64, NQ * j + c0:NQ * (j + 1)],
                    start=True, stop=True)
                nc.tensor.matmul(
                    s[:, NQ + c0:2 * NQ],
                    kT[p][64:128, 128 * ci:128 * (ci + 1)],
                    qT[p][64:128, NQ * j + c0:NQ * (j + 1)],
                    start=True, stop=True)
                di = ci - 4 * j
                if di >= 0:
                    sv = s.rearrange("p (h x) -> p h x", h=2)[
                        :, :, c0:c0 + 128]
                    nc.vector.tensor_add(
                        sv, sv,
                        maskadd.rearrange("p (h x) -> p h x", h=2))
                return s

            chunks = []         # flat stream: (p, j, ci, nchunks)
            hooks = {}          # (p, j) -> after_j hook
            for p in range(3):
                for j in range(NTQ):
                    n = 4 * (j + 1)
                    for ci in range(n):
                        chunks.append((p, j, ci, n))

            def attn_all():
                accs = {}
                s_tiles = {}
                first = chunks[0]
                s_tiles[first[:3]] = s_emit(*first[:3])
                for idx, (p, j, ci, n) in enumerate(chunks):
                    if ci == 0:
                        acc0 = ps_o.tile([128, NQ], dt.float32, tag="acc",
                                         name="acc0")
                        acc1 = ps_o.tile([128, NQ], dt.float32, tag="acc",
                                         name="acc1")
                        accs[(p, j)] = (acc0, acc1)
                    acc0, acc1 = accs[(p, j)]
                    s = s_tiles.pop((p, j, ci))
                    pt = ptp.tile([128, 2 * NQ], dt.bfloat16, tag="pt",
                                  name="pt")
                    # cols [0, 128*di) of a diagonal chunk are fully masked:
                    # exp and the PV matmuls skip them entirely
                    c0 = max(ci - 4 * j, 0) * 128
                    if c0 == 0:
                        nc.scalar.activation(
                            pt, s, mybir.ActivationFunctionType.Exp,
                            bias=0.0, scale=0.125)
                    else:
                        pv = pt.rearrange("p (h x) -> p h x", h=2)[
                            :, :, c0:NQ]
                        sv = s.rearrange("p (h x) -> p h x", h=2)[
                            :, :, c0:NQ]
                        nc.scalar.activation(
                            pv, sv, mybir.ActivationFunctionType.Exp,
                            bias=0.0, scale=0.125)
                    if idx + 1 < len(chunks):
                        nxt = chunks[idx + 1]
                        s_tiles[nxt[:3]] = s_emit(*nxt[:3])
                    last = (ci == n - 1)
                    nc.tensor.matmul(
                        acc0[0:65, c0:NQ],
                        vn[p][:, 130 * ci:130 * ci + 65],
                        pt[:, c0:NQ], start=(ci == 0), stop=last)
                    nc.tensor.matmul(
                        acc1[0:65, c0:NQ],
                        vn[p][:, 130 * ci + 65:130 * ci + 130],
                        pt[:, NQ + c0:2 * NQ], start=(ci == 0), stop=last)
                    slot_cb()
                    if last:
                        for h, acc in enumerate((acc0, acc1)):
                            nc.vector.tensor_copy(
                                oT[p][64 * h:64 * h + 64,
                                      NQ * j:NQ * (j + 1)],
                                acc[0:64, :])
                            r, c = drow(j, h)
                            nc.vector.tensor_copy(
                                dens[p][r:r + 1, c:c + NQ], acc[64:65, :])
                        accs.pop((p, j))
                        hk = hooks.get((p, j))
                        if hk is not None:
                            hk()

            # norm/y filler schedule (absolute slots: pair p spans
            # [40p, 40p+40)); normalization split per j-half so y units
            # spread across pairs instead of bunching in pair 2
            hooks[(0, 1)] = lambda: fillers.extend(
                norm_fillers(0, 13, [0, 1]))
            hooks[(0, 3)] = lambda: fillers.extend(
                norm_fillers(0, 41, [2, 3]))

            def queue_p1_half1():
                fillers.extend(norm_fillers(1, 53, [0, 1]))
                for m in range(8):
                    fillers.append((59 + m, y_unit(y0, (0, 1), m)))
            hooks[(1, 1)] = queue_p1_half1

            def queue_p1_half2():
                fillers.extend(norm_fillers(1, 81, [2, 3]))
                for m in range(8, 16):
                    fillers.append((79 + m, y_unit(y0, (0, 1), m)))
            hooks[(1, 3)] = queue_p1_half2

            def queue_p2_half1():
                fillers.extend(norm_fillers(2, 93, [0, 1]))
                for m in range(8):
                    fillers.append((99 + m, y_unit(y1, (2,), m)))
            hooks[(2, 1)] = queue_p2_half1

            def queue_p2_mid():
                fillers.extend(norm_fillers(2, 107, [2]))
                for m in range(8, 12):
                    fillers.append((111 + m, y_unit(y1, (2,), m)))
            hooks[(2, 2)] = queue_p2_mid

            make_projB_fillers()
            attn_all()
            drain_fillers()
            # tail: j3 norm + last y1 units
            rb2s_t = {}
            recip_batch(2, [6, 7])()
            norm_bc_unit(2, 3, rb2s_t)()
            norm_mul_unit(2, 3, rb2s_t)()
            for m in range(12, 16):
                y_unit(y1, (2,), m, cast_on_act=True)()

    nc.compile()
    return nc


def _get_program():
    if "nc" not in _CACHE:
        _CACHE["nc"] = _build_program()
    return _CACHE["nc"]


def _sbuf_layout(xT):
    """[1024, 2048] (e, pos) -> [128, 2*8*1024]: pos-half major, ec, pos."""
    # [ec, 128, t2, 1024] -> [128, t2, ec, 1024]
    return np.ascontiguousarray(
        xT.reshape(ECH, 128, 2, 1024).transpose(1, 2, 0, 3).reshape(
            128, 2 * ECH * 1024))


def _prep_inputs(query, key, value, Wq, bq, Wk, bk, Wv, bv, Wo, bo):
    """Build the 8 per-core input maps (host-side slicing + bf16 cast)."""
    q = np.asarray(query, np.float32).reshape(8192, 1024).astype(BF16)
    k = np.asarray(key, np.float32).reshape(8192, 1024).astype(BF16)
    v = np.asarray(value, np.float32).reshape(8192, 1024).astype(BF16)
    wq = np.asarray(Wq, np.float32).astype(BF16)
    wk = np.asarray(Wk, np.float32).astype(BF16)
    wv = np.asarray(Wv, np.float32).astype(BF16)
    wo_f = np.asarray(Wo, np.float32).astype(BF16)
    bqf = np.asarray(bq, np.float32)
    bkf = np.asarray(bk, np.float32)
    bvf = np.asarray(bv, np.float32)

    qT, kT, vT = q.T, k.T, v.T  # [1024, 8192] views
    in_maps = []
    for c in range(8):
        a, sc, b, rc = c % 2, c // 2, c % 4, c // 4
        rows_g0 = slice(2048 * sc, 2048 * (sc + 1))
        rows_g1 = slice(4096 * rc + 1, 4096 * (rc + 1), 2)
        hrows = np.r_[256 * a:256 * a + 256, 512 + 128 * b:512 + 128 * b + 128]

        def wlay(w):
            # [1024, 384] -> [128, 8*384] (ec-blocked)
            return np.ascontiguousarray(
                w.reshape(ECH, 128, 384).transpose(1, 0, 2).reshape(128, -1))

        wov = np.ascontiguousarray(wo_f[:, hrows].T)  # [384, 1024]
        ballv = np.stack([np.ascontiguousarray(bf[hrows]).reshape(3, 128)
                          for bf in (bqf, bkf, bvf)], axis=0)  # [3,3,128]
        # ball[128, 9]: col 3i+p = input i, pair p
        ballv = np.ascontiguousarray(ballv.reshape(9, 128).T)
        m = {
            "xa_q": _sbuf_layout(np.ascontiguousarray(qT[:, rows_g0])),
            "xa_k": _sbuf_layout(np.ascontiguousarray(kT[:, rows_g0])),
            "xa_v": _sbuf_layout(np.ascontiguousarray(vT[:, rows_g0])),
            "xb_q": _sbuf_layout(np.ascontiguousarray(qT[:, rows_g1])),
            "xb_k": _sbuf_layout(np.ascontiguousarray(kT[:, rows_g1])),
            "xb_v": _sbuf_layout(np.ascontiguousarray(vT[:, rows_g1])),
            "wq": wlay(np.ascontiguousarray(wq[hrows].T)),
            "wk": wlay(np.ascontiguousarray(wk[hrows].T)),
            "wv": wlay(np.ascontiguousarray(wv[hrows].T)),
            "wo": np.ascontiguousarray(
                wov.reshape(3, 128, 1024).transpose(1, 0, 2).reshape(128, -1)),
            "ball": ballv,
        }
        in_maps.append(m)
    return in_maps


def _combine(results, bo):
    y = np.zeros((8192, 1024), np.float32)
    for c in range(8):
        sc, rc = c // 2, c // 4
        y[2048 * sc:2048 * (sc + 1)] += results[c]["y0"].astype(np.float32)
        y[4096 * rc + 1:4096 * (rc + 1):2] += results[c]["y1"].astype(np.float32)
    y += np.asarray(bo, np.float32)
    return y.reshape(1, 8192, 1024)


def kernel(query, key, value, Wq, bq, Wk, bk, Wv, bv, Wo, bo,
           _trace=False, _trace_cores=None):
    from concourse import bass_utils
    nc = _get_program()
    in_maps = _prep_inputs(query, key, value, Wq, bq, Wk, bk, Wv, bv, Wo, bo)
    res = bass_utils.run_bass_kernel_spmd(
        nc, in_maps, core_ids=list(range(8)),
        trace=_trace, trace_cores=_trace_cores)
    _CACHE["last_results"] = res
    return _combine(res.results, bo)
